# revision 11
# baseline (speedup 1.0000x reference)
"""DeepseekV3 MLA forward on 8 TRN2 NeuronCores.

Sharding: data-parallel over batch (B=2 -> 2 groups of 4 cores), tensor-
parallel over heads within each batch group (32 heads -> 4 groups of 8).
Each core computes its batch element's full latent projections (wq_a /
wkv_a replicated), its 8 heads' q/k/v expansions + attention, and a
partial output projection (wo row-shard); the host sums the 4 partial
outputs per batch element.

Dataflow on device keeps activations transposed ([feature, token]) so
every matmul contracts over the partition dim with no on-device
transposes anywhere:
  latentT = wq_a.T @ xT        (lhsT=wq_a cols, rhs=xT)        [rank, T]
  qT_h    = wq_b_h.T @ latentT                                  [d, T]
  scoresT = kT_h-chunks @ qT_h                                  [tk, tq]
  softmax over tk (=partitions): exp on ACT, denominator via a
  ones[128,128] matmul (result replicated across partitions), then
  attn_outT = v_chunks.T @ expT                                 [dv, tq]
  out      = attnT-chunks.T @ wo_h  (natural layout)            [tq, hid]
RoPE in transposed layout: rot(x) = x*cos + swap32(x)*(+-sin), where
swap32 exchanges the two 32-row halves of each 64-row rope block (done
with SBUF->SBUF block DMAs) and the +-sin sign pattern is host-built.
All matmuls run as float32r (FP22-truncated fp32, single pass).
SBUF pools are statically reserved, so pools are scoped per stage.
"""

import os
import sys

import numpy as np

sys.path.insert(0, "/opt/trn_rl_repo")

B, T, HID = 2, 1024, 4096
H, D_NOPE, D_ROPE, D_V = 32, 128, 64, 128
D_QK = D_NOPE + D_ROPE
Q_RANK, KV_RANK = 1536, 512
THETA, EPS = 10000.0, 1e-6
SCALE = float(D_QK) ** -0.5
NMASK = -30000.0

HG = H // 4          # heads per core = 8
P = 128
QCH = Q_RANK // P    # 12 latent chunks (q)
KCH = KV_RANK // P   # 4 latent chunks (kv)
HIDK = HID // P      # 32 contraction tiles for stage A
TQ = 512             # token tile (free dim) for most matmuls
NT = T // TQ         # 2 token tiles
TC = T // P          # 8 token chunks of 128

_CACHED = {}


def _build_program():
    import concourse.bacc as bacc
    import concourse.mybir as mybir
    import concourse.tile as tile

    f32 = mybir.dt.float32
    f32r = mybir.dt.float32r
    AF = mybir.ActivationFunctionType
    ALU = mybir.AluOpType

    nc = bacc.Bacc()

    # ---- DRAM I/O (per-core shapes; SPMD across the 8 cores) ----
    xT = nc.dram_tensor("xT", (HID, T), f32, kind="ExternalInput")
    wqa = nc.dram_tensor("wqa", (HID, Q_RANK), f32, kind="ExternalInput")
    wkva = nc.dram_tensor("wkva", (HID, KV_RANK + D_ROPE), f32, kind="ExternalInput")
    wqbn = nc.dram_tensor("wqbn", (Q_RANK, HG * D_NOPE), f32, kind="ExternalInput")
    wqbr = nc.dram_tensor("wqbr", (Q_RANK, HG * D_ROPE), f32, kind="ExternalInput")
    wkbn = nc.dram_tensor("wkbn", (KV_RANK, HG * D_NOPE), f32, kind="ExternalInput")
    wkbv = nc.dram_tensor("wkbv", (KV_RANK, HG * D_V), f32, kind="ExternalInput")
    wo = nc.dram_tensor("wo", (HG * D_V, HID), f32, kind="ExternalInput")
    qw = nc.dram_tensor("qw", (P, QCH), f32, kind="ExternalInput")
    kvw = nc.dram_tensor("kvw", (P, KCH), f32, kind="ExternalInput")
    cos4 = nc.dram_tensor("cos4", (P, T), f32, kind="ExternalInput")
    sin4 = nc.dram_tensor("sin4", (P, T), f32, kind="ExternalInput")  # +-sin
    cmask = nc.dram_tensor("cmask", (P, 4, TQ), f32, kind="ExternalInput")
    kbias = nc.dram_tensor("kbias", (P, TC), f32, kind="ExternalInput")
    onesd = nc.dram_tensor("onesd", (P, P), f32, kind="ExternalInput")
    out = nc.dram_tensor("out", (T, HID), f32, kind="ExternalOutput")

    def r(ap):
        return ap.bitcast(f32r)

    import contextlib

    with tile.TileContext(nc) as tc, contextlib.ExitStack() as rstack:
        with (
            tc.tile_pool(name="const", bufs=1) as const,
            tc.tile_pool(name="psmm", bufs=4, space="PSUM") as psum,
            tc.tile_pool(name="pspd", bufs=1, space="PSUM") as pspd,
            tc.tile_pool(name="psav", bufs=1, space="PSUM") as psav,
            tc.tile_pool(name="pssc", bufs=2, space="PSUM") as pssc,
        ):
            # ---- constants (persistent) ----
            ones_sb = const.tile([P, P], f32r, tag="ones")
            nc.sync.dma_start(ones_sb[:], onesd[:, :].bitcast(f32r))
            qw_sb = const.tile([P, QCH], f32, tag="qw")
            nc.sync.dma_start(qw_sb[:], qw[:, :])
            kvw_sb = const.tile([P, KCH], f32, tag="kvw")
            nc.sync.dma_start(kvw_sb[:], kvw[:, :])
            cos_sb = const.tile([P, T], f32, tag="cos")
            nc.sync.dma_start(cos_sb[:], cos4[:, :])
            sin_sb = const.tile([P, T], f32, tag="sin")
            nc.sync.dma_start(sin_sb[:], sin4[:, :])
            cm_sb = const.tile([P, 4, TQ], f32, tag="cm")
            nc.sync.dma_start(cm_sb[:], cmask[:, :, :])
            kb_sb = const.tile([P, TC], f32, tag="kb")
            nc.sync.dma_start(kb_sb[:], kbias[:, :])
            zero_b = const.tile([P, 1], f32, tag="zb")
            nc.vector.memset(zero_b[:], 0.0)
            eps_b = const.tile([P, 1], f32, tag="eb")
            nc.vector.memset(eps_b[:], EPS)

            with tc.tile_pool(name="kvlatp", bufs=1) as kvlatp:
                kvlat = kvlatp.tile([P, KCH + 1, T], f32r, tag="kvlat")
                with tc.tile_pool(name="qlatp", bufs=1) as qlatp:
                    qlat = qlatp.tile([P, QCH, T], f32r, tag="qlat")

                    # ---- stage A: latent projections (contract over HID) --
                    mblocks = []
                    for mb in range(QCH // 2):
                        mblocks.append((wqa, mb * 256, 256, qlat, 2 * mb))
                    mblocks.append((wkva, 0, 256, kvlat, 0))
                    mblocks.append((wkva, 256, 256, kvlat, 2))
                    mblocks.append((wkva, 512, 64, kvlat, 4))

                    KH = HIDK // 2  # 16 k-tiles per half
                    with (
                        tc.tile_pool(name="xk", bufs=18) as xkp,
                        tc.tile_pool(name="wA", bufs=6) as wAp,
                    ):
                        for khalf in range(2):
                            xk = []
                            for kk in range(KH):
                                k = khalf * KH + kk
                                xt_ = xkp.tile([P, T], f32r, tag="xk", name="xk")
                                nc.sync.dma_start(xt_[:], xT[k * P : (k + 1) * P, :].bitcast(f32r))
                                xk.append(xt_)
                            for wdram, coff, width, dest, dch in mblocks:
                                nm = (width + P - 1) // P
                                pst = [
                                    [
                                        psum.tile([P, TQ], f32, tag="mm", name="psA")
                                        for _ in range(NT)
                                    ]
                                    for _ in range(nm)
                                ]
                                for kk in range(KH):
                                    k = khalf * KH + kk
                                    wt = wAp.tile([P, 256], f32r, tag="wA", name="wA")
                                    nc.sync.dma_start(
                                        wt[:, :width],
                                        wdram[k * P : (k + 1) * P, coff : coff + width].bitcast(f32r),
                                    )
                                    for m in range(nm):
                                        w = min(width - m * P, P)
                                        for t in range(NT):
                                            nc.tensor.matmul(
                                                pst[m][t][:w, :],
                                                r(wt[:, m * P : m * P + w]),
                                                r(xk[kk][:, t * TQ : (t + 1) * TQ]),
                                                start=(kk == 0),
                                                stop=(kk == KH - 1),
                                            )
                                for m in range(nm):
                                    w = min(width - m * P, P)
                                    for t in range(NT):
                                        dst = dest[:w, dch + m, t * TQ : (t + 1) * TQ]
                                        if khalf == 0:
                                            nc.vector.tensor_copy(dst, pst[m][t][:w, :])
                                        else:
                                            nc.vector.tensor_tensor(
                                                dst, pst[m][t][:w, :], dst, ALU.add
                                            )

                    # ---- stage A2: RMSNorm on latents (in place) ----
                    with tc.tile_pool(name="wrkA", bufs=3) as wrkA:
                        for lat, nch, w_sb, fan in (
                            (qlat, QCH, qw_sb, Q_RANK),
                            (kvlat, KCH, kvw_sb, KV_RANK),
                        ):
                            for t in range(NT):
                                ssp = psum.tile([P, TQ], f32, tag="mm", name="ssp")
                                for m in range(nch):
                                    sq = wrkA.tile([P, TQ], f32r, tag="sq", name="sq")
                                    nc.scalar.activation(
                                        sq[:],
                                        lat[:, m, t * TQ : (t + 1) * TQ],
                                        AF.Square,
                                        bias=zero_b[:],
                                    )
                                    nc.tensor.matmul(
                                        ssp[:],
                                        r(ones_sb[:]),
                                        r(sq[:]),
                                        start=(m == 0),
                                        stop=(m == nch - 1),
                                    )
                                std = wrkA.tile([P, TQ], f32, tag="std", name="std")
                                nc.scalar.activation(
                                    std[:], ssp[:], AF.Sqrt, bias=eps_b[:],
                                    scale=1.0 / fan,
                                )
                                rstd = wrkA.tile([P, TQ], f32, tag="rstd", name="rstd")
                                with nc.allow_low_precision("rmsnorm rstd"):
                                    nc.vector.reciprocal(rstd[:], std[:])
                                for m in range(nch):
                                    sl = lat[:, m, t * TQ : (t + 1) * TQ]
                                    nc.vector.tensor_tensor(sl, sl, rstd[:], ALU.mult)
                                    nc.vector.tensor_scalar_mul(
                                        sl, sl, w_sb[:, m : m + 1]
                                    )

                    # right-side pools: live until the end of the kernel
                    actk = rstack.enter_context(
                        tc.tile_pool(name="actk", bufs=1, side="right"))
                    kTn = actk.tile([P, HG, T], f32r, tag="kTn")
                    kpe2 = actk.tile([P, T], f32r, tag="kpe2")

                    # ---- k_pe rope -> kpe2 (both 64-row halves identical) --
                    with tc.tile_pool(name="ropek", bufs=1) as ropek:
                        ksw = ropek.tile([P, T], f32, tag="ksw")
                        nc.sync.dma_start(kpe2[0:64, :], kvlat[0:64, KCH, :])
                        nc.sync.dma_start(kpe2[64:128, :], kvlat[0:64, KCH, :])
                        nc.sync.dma_start(ksw[0:32, :], kvlat[32:64, KCH, :].bitcast(f32))
                        nc.sync.dma_start(ksw[32:64, :], kvlat[0:32, KCH, :].bitcast(f32))
                        nc.sync.dma_start(ksw[64:96, :], kvlat[32:64, KCH, :].bitcast(f32))
                        nc.sync.dma_start(ksw[96:128, :], kvlat[0:32, KCH, :].bitcast(f32))
                        nc.vector.tensor_tensor(ksw[:], ksw[:], sin_sb[:], ALU.mult)
                        nc.vector.tensor_tensor(
                            kpe2[:], kpe2[:], cos_sb[:], ALU.mult
                        )
                        nc.vector.tensor_tensor(kpe2[:], kpe2[:], ksw[:], ALU.add)

                    actq = rstack.enter_context(
                        tc.tile_pool(name="actq", bufs=1, side="right"))
                    qTn = actq.tile([P, HG, T], f32r, tag="qTn")
                    qTr = actq.tile([P, HG // 2, T], f32r, tag="qTr")

                    # ---- stage B: qT per head ----
                    with tc.tile_pool(name="wqb", bufs=6) as wqbp:
                        for h in range(HG):
                            pst = [
                                psum.tile([P, TQ], f32, tag="mm", name="psB")
                                for _ in range(NT)
                            ]
                            for k in range(QCH):
                                wt = wqbp.tile([P, P], f32r, tag="wqbn", name="wqbn")
                                nc.sync.dma_start(
                                    wt[:],
                                    wqbn[k * P : (k + 1) * P, h * P : (h + 1) * P].bitcast(f32r),
                                )
                                for t in range(NT):
                                    nc.tensor.matmul(
                                        pst[t][:],
                                        r(wt[:]),
                                        r(qlat[:, k, t * TQ : (t + 1) * TQ]),
                                        start=(k == 0),
                                        stop=(k == QCH - 1),
                                    )
                            for t in range(NT):
                                nc.vector.tensor_copy(
                                    qTn[:, h, t * TQ : (t + 1) * TQ], pst[t][:]
                                )
                        for p_ in range(HG // 2):
                            pst = [
                                psum.tile([P, TQ], f32, tag="mm", name="psB2")
                                for _ in range(NT)
                            ]
                            for k in range(QCH):
                                wt = wqbp.tile([P, P], f32r, tag="wqbr", name="wqbr")
                                nc.sync.dma_start(
                                    wt[:],
                                    wqbr[k * P : (k + 1) * P, p_ * P : (p_ + 1) * P].bitcast(f32r),
                                )
                                for t in range(NT):
                                    nc.tensor.matmul(
                                        pst[t][:],
                                        r(wt[:]),
                                        r(qlat[:, k, t * TQ : (t + 1) * TQ]),
                                        start=(k == 0),
                                        stop=(k == QCH - 1),
                                    )
                            for t in range(NT):
                                nc.vector.tensor_copy(
                                    qTr[:, p_, t * TQ : (t + 1) * TQ], pst[t][:]
                                )

                # qlat pool closed here
                # rope on qTr (pairs: head-even rows 0:64, head-odd 64:128)
                with tc.tile_pool(name="ropeq", bufs=1) as ropeq:
                    qsw = ropeq.tile([P, HG // 2, T], f32, tag="qsw")
                    nc.sync.dma_start(qsw[0:32, :, :], qTr[32:64, :, :].bitcast(f32))
                    nc.sync.dma_start(qsw[32:64, :, :], qTr[0:32, :, :].bitcast(f32))
                    nc.sync.dma_start(qsw[64:96, :, :], qTr[96:128, :, :].bitcast(f32))
                    nc.sync.dma_start(qsw[96:128, :, :], qTr[64:96, :, :].bitcast(f32))
                    cosb = cos_sb[:, None, :].to_broadcast((P, HG // 2, T))
                    sinb = sin_sb[:, None, :].to_broadcast((P, HG // 2, T))
                    nc.vector.tensor_tensor(qsw[:], qsw[:], sinb, ALU.mult)
                    nc.vector.tensor_tensor(qTr[:], qTr[:], cosb, ALU.mult)
                    nc.vector.tensor_tensor(qTr[:], qTr[:], qsw[:], ALU.add)

                vqp = rstack.enter_context(
                    tc.tile_pool(name="vqp", bufs=2, side="right"))
                vq = [
                    vqp.tile([P, TC, 4 * D_V], f32r, tag="vq", name="vq")
                    for _ in range(2)
                ]

                # ---- stage D: kT_nope per head, v (natural) per quad ----
                with tc.tile_pool(name="wkb", bufs=6) as wkbp:
                    for h in range(HG):
                        pst = [
                            psum.tile([P, TQ], f32, tag="mm", name="psD")
                            for _ in range(NT)
                        ]
                        for k in range(KCH):
                            wt = wkbp.tile([P, P], f32r, tag="wkbn", name="wkbn")
                            nc.sync.dma_start(
                                wt[:],
                                wkbn[k * P : (k + 1) * P, h * P : (h + 1) * P].bitcast(f32r),
                            )
                            for t in range(NT):
                                nc.tensor.matmul(
                                    pst[t][:],
                                    r(wt[:]),
                                    r(kvlat[:, k, t * TQ : (t + 1) * TQ]),
                                    start=(k == 0),
                                    stop=(k == KCH - 1),
                                )
                        for t in range(NT):
                            nc.vector.tensor_copy(
                                kTn[:, h, t * TQ : (t + 1) * TQ], pst[t][:]
                            )
                    for quad in range(2):
                        for tkc in range(TC):
                            ps_ = psum.tile([P, TQ], f32, tag="mm", name="psV")
                            for k in range(KCH):
                                wt = wkbp.tile(
                                    [P, TQ], f32r, tag="wkbv", name="wkbv"
                                )
                                nc.sync.dma_start(
                                    wt[:],
                                    wkbv[
                                        k * P : (k + 1) * P,
                                        quad * TQ : (quad + 1) * TQ,
                                    ].bitcast(f32r),
                                )
                                nc.tensor.matmul(
                                    ps_[:],
                                    r(kvlat[:, k, tkc * P : (tkc + 1) * P]),
                                    r(wt[:]),
                                    start=(k == 0),
                                    stop=(k == KCH - 1),
                                )
                            nc.vector.tensor_copy(vq[quad][:, tkc, :], ps_[:])

            # kvlat pool closed here
            # ---- stages E+F per token tile ----
            with (
                tc.tile_pool(name="attp", bufs=1) as attp,
                tc.tile_pool(name="wrkE", bufs=3) as wrkE,
                tc.tile_pool(name="wop", bufs=8) as wop,
                tc.tile_pool(name="outs", bufs=3) as outp,
            ):
                for t in range(NT):
                    attnT = attp.tile(
                        [P, HG, TQ], f32r, tag="attnT", name="attnT"
                    )
                    nchunks = 4 * (t + 1)
                    for h in range(HG):
                        hb = 64 * (h % 2)
                        pd = pspd.tile([P, TQ], f32, tag="pd", name="pd")
                        pav = psav.tile([P, TQ], f32, tag="pav", name="pav")
                        for tkc in range(nchunks):
                            ps_ = pssc.tile([P, TQ], f32, tag="psc", name="psc")
                            nc.tensor.matmul(
                                ps_[:],
                                r(kTn[:, h, tkc * P : (tkc + 1) * P]),
                                r(qTn[:, h, t * TQ : (t + 1) * TQ]),
                                start=True,
                                stop=False,
                            )
                            nc.tensor.matmul(
                                ps_[:],
                                r(kpe2[hb : hb + 64, tkc * P : (tkc + 1) * P]),
                                r(
                                    qTr[
                                        hb : hb + 64,
                                        h // 2,
                                        t * TQ : (t + 1) * TQ,
                                    ]
                                ),
                                start=False,
                                stop=True,
                            )
                            if tkc >= 4 * t:
                                nc.vector.tensor_tensor(
                                    ps_[:],
                                    ps_[:],
                                    cm_sb[:, tkc - 4 * t, :],
                                    ALU.add,
                                )
                            ex = wrkE.tile([P, TQ], f32r, tag="exp", name="ex")
                            nc.scalar.activation(
                                ex[:],
                                ps_[:],
                                AF.Exp,
                                bias=kb_sb[:, tkc : tkc + 1],
                                scale=SCALE,
                            )
                            nc.tensor.matmul(
                                pd[:],
                                r(ones_sb[:]),
                                r(ex[:]),
                                start=(tkc == 0),
                                stop=(tkc == nchunks - 1),
                            )
                            nc.tensor.matmul(
                                pav[:],
                                r(
                                    vq[h // 4][
                                        :, tkc, (h % 4) * P : (h % 4 + 1) * P
                                    ]
                                ),
                                r(ex[:]),
                                start=(tkc == 0),
                                stop=(tkc == nchunks - 1),
                            )
                        rec = wrkE.tile([P, TQ], f32, tag="rec", name="rec")
                        with nc.allow_low_precision("softmax denom"):
                            nc.vector.reciprocal(rec[:], pd[:])
                        nc.vector.tensor_tensor(
                            attnT[:, h, :], pav[:], rec[:], ALU.mult
                        )
                    # output projection for this token tile
                    for nt in range(HID // TQ):
                        wts = []
                        for h in range(HG):
                            wt = wop.tile([P, TQ], f32r, tag="wo", name="wo")
                            nc.sync.dma_start(
                                wt[:],
                                wo[
                                    h * P : (h + 1) * P,
                                    nt * TQ : (nt + 1) * TQ,
                                ].bitcast(f32r),
                            )
                            wts.append(wt)
                        for tqc in range(TQ // P):
                            po = psum.tile([P, TQ], f32, tag="mm", name="po")
                            for h in range(HG):
                                nc.tensor.matmul(
                                    po[:],
                                    r(attnT[:, h, tqc * P : (tqc + 1) * P]),
                                    r(wts[h][:]),
                                    start=(h == 0),
                                    stop=(h == HG - 1),
                                )
                            ot = outp.tile([P, TQ], f32, tag="osb", name="ot")
                            nc.vector.tensor_copy(ot[:], po[:])
                            row0 = t * TQ + tqc * P
                            nc.sync.dma_start(
                                out[row0 : row0 + P, nt * TQ : (nt + 1) * TQ],
                                ot[:],
                            )

    nc.finalize()
    return nc


def _get_program():
    if "nc" not in _CACHED:
        _CACHED["nc"] = _build_program()
    return _CACHED["nc"]


def _host_prep(x, wq_a, q_norm_w, wq_b, wkv_a, kv_norm_w, wkv_b, wo,
               attention_mask, positions):
    """Build the 8 per-core input maps (cheap numpy slicing/transposes)."""
    f = np.float32
    x = np.asarray(x, f)
    wq_a = np.ascontiguousarray(np.asarray(wq_a, f))
    wkv_a = np.ascontiguousarray(np.asarray(wkv_a, f))
    wq_b3 = np.asarray(wq_b, f).reshape(Q_RANK, H, D_QK)
    wkv_b3 = np.asarray(wkv_b, f).reshape(KV_RANK, H, D_NOPE + D_V)
    wo2 = np.asarray(wo, f)
    q_norm_w = np.asarray(q_norm_w, f)
    kv_norm_w = np.asarray(kv_norm_w, f)
    attention_mask = np.asarray(attention_mask)
    positions = np.asarray(positions)

    qw = np.ascontiguousarray(q_norm_w.reshape(QCH, P).T)
    kvw = np.ascontiguousarray(kv_norm_w.reshape(KCH, P).T)

    inv_freq = 1.0 / (THETA ** (np.arange(0, D_ROPE, 2, dtype=np.float64) / D_ROPE))

    # causal additive mask variants r = tk_chunk_base - tq_tile_base
    dk = np.arange(P)[:, None, None]
    ri = np.arange(4)[None, :, None]
    dq = np.arange(TQ)[None, None, :]
    cmask = np.where(dk + ri * P <= dq, 0.0, NMASK).astype(f)
    onesd = np.ones((P, P), f)

    per_batch = {}
    for b in range(B):
        xTb = np.ascontiguousarray(x[b].T)
        ang = positions[b].astype(np.float64)[:, None] * inv_freq[None, :]
        cosT = np.cos(ang).astype(f).T  # [32, T]
        sinT = np.sin(ang).astype(f).T
        cos4 = np.ascontiguousarray(np.tile(cosT, (4, 1)))
        sin4 = np.ascontiguousarray(
            np.concatenate([-sinT, sinT, -sinT, sinT], axis=0))
        kb = np.where(attention_mask[b] != 0, 0.0, NMASK).astype(f)
        kbias = np.ascontiguousarray(kb.reshape(TC, P).T)
        per_batch[b] = (xTb, cos4, sin4, kbias)

    in_maps = []
    for c in range(8):
        b, g = c // 4, c % 4
        hs = slice(g * HG, (g + 1) * HG)
        xTb, cos4, sin4, kbias = per_batch[b]
        wqbn = np.ascontiguousarray(
            wq_b3[:, hs, :D_NOPE].reshape(Q_RANK, HG * D_NOPE))
        # rope cols packed in head pairs: [h_even 64 | h_odd 64] per 128-col
        wqbr = np.ascontiguousarray(
            wq_b3[:, hs, D_NOPE:].reshape(Q_RANK, HG * D_ROPE))
        wkbn = np.ascontiguousarray(
            wkv_b3[:, hs, :D_NOPE].reshape(KV_RANK, HG * D_NOPE))
        wkbv = np.ascontiguousarray(
            wkv_b3[:, hs, D_NOPE:].reshape(KV_RANK, HG * D_V))
        wosh = np.ascontiguousarray(wo2[g * HG * D_V : (g + 1) * HG * D_V, :])
        in_maps.append({
            "xT": xTb, "wqa": wq_a, "wkva": wkv_a,
            "wqbn": wqbn, "wqbr": wqbr, "wkbn": wkbn, "wkbv": wkbv,
            "wo": wosh, "qw": qw, "kvw": kvw,
            "cos4": cos4, "sin4": sin4,
            "cmask": cmask, "kbias": kbias, "onesd": onesd,
        })
    return in_maps


def kernel(**inputs):
    from concourse.bass_utils import run_bass_kernel_spmd

    nc = _get_program()
    in_maps = _host_prep(**inputs)
    res = run_bass_kernel_spmd(nc, in_maps, core_ids=list(range(8)))
    _CACHED["last_result"] = res
    out = np.zeros((B, T, HID), np.float32)
    for c in range(8):
        out[c // 4] += res.results[c]["out"]
    return out


# revision 21
# speedup vs baseline: 1.2450x; 1.2450x over previous
"""DeepseekV3 MLA forward on 8 TRN2 NeuronCores.

Sharding: data-parallel over batch (B=2 -> 2 groups of 4 cores), tensor-
parallel over heads within each batch group (32 heads -> 4 groups of 8).
Each core computes its batch element's full latent projections (wq_a /
wkv_a replicated), its 8 heads' q/k/v expansions + attention, and a
partial output projection (wo row-shard); the host sums the 4 partial
outputs per batch element.

Dataflow on device keeps activations transposed ([feature, token]) so
every matmul contracts over the partition dim with no on-device
transposes anywhere:
  latentT = wq_a.T @ xT        (lhsT=wq_a cols, rhs=xT)        [rank, T]
  qT_h    = wq_b_h.T @ latentT                                  [d, T]
  scoresT = kT_h-chunks @ qT_h                                  [tk, tq]
  softmax over tk (=partitions): exp on ACT, denominator via a
  ones[128,128] matmul (result replicated across partitions), then
  attn_outT = v_chunks.T @ expT                                 [dv, tq]
  out      = attnT-chunks.T @ wo_h  (natural layout)            [tq, hid]
RoPE in transposed layout: rot(x) = x*cos + swap32(x)*(+-sin), where
swap32 exchanges the two 32-row halves of each 64-row rope block (done
with SBUF->SBUF block DMAs) and the +-sin sign pattern is host-built.
All matmuls run as float32r (FP22-truncated fp32, single pass).

Perf notes: per-DMA issue costs ~1us on the sync sequencer, so weights
are host-packed into large partition-major blocks and DMA'd in few big
transfers (SBUF slices feed the matmuls). SBUF pools are statically
reserved; they are scoped per stage, long-lived activation pools on the
"right" allocator side. PSUM: 8 banks split mm:4 / pd:2 / sc:2.
"""

import os
import sys

import numpy as np

sys.path.insert(0, "/opt/trn_rl_repo")

B, T, HID = 2, 1024, 4096
H, D_NOPE, D_ROPE, D_V = 32, 128, 64, 128
D_QK = D_NOPE + D_ROPE
Q_RANK, KV_RANK = 1536, 512
THETA, EPS = 10000.0, 1e-6
SCALE = float(D_QK) ** -0.5
NMASK = -30000.0

HG = H // 4          # heads per core = 8
P = 128
QCH = Q_RANK // P    # 12 latent chunks (q)
KCH = KV_RANK // P   # 4 latent chunks (kv)
HIDK = HID // P      # 32 contraction tiles for stage A
KQ = HIDK // 4       # 8 k-tiles per stage-A quarter
TQ = 512             # token tile (free dim) for most matmuls
NT = T // TQ         # 2 token tiles
TC = T // P          # 8 token chunks of 128
NHID = HID // TQ     # 8 output column tiles

_CACHED = {}
STAGE_MARKS = []


def _build_program():
    import contextlib

    import concourse.bacc as bacc
    import concourse.mybir as mybir
    import concourse.tile as tile

    f32 = mybir.dt.float32
    f32r = mybir.dt.float32r
    AF = mybir.ActivationFunctionType
    ALU = mybir.AluOpType

    nc = bacc.Bacc()

    # ---- DRAM I/O (per-core shapes; SPMD across the 8 cores) ----
    # weights are host-packed partition-major so each DMA is one big
    # contiguous transfer whose SBUF image is sliced per matmul
    xT = nc.dram_tensor("xT", (HID, T), f32, kind="ExternalInput")
    wqa = nc.dram_tensor("wqa", (6, 4, P, KQ, 256), f32, kind="ExternalInput")
    wkva = nc.dram_tensor("wkva", (2, 4, P, KQ, 256), f32, kind="ExternalInput")
    wkvr = nc.dram_tensor("wkvr", (4, P, KQ, 64), f32, kind="ExternalInput")
    wqbn = nc.dram_tensor("wqbn", (HG, P, QCH, P), f32, kind="ExternalInput")
    wqbr = nc.dram_tensor("wqbr", (HG // 2, P, QCH, P), f32, kind="ExternalInput")
    wkbn = nc.dram_tensor("wkbn", (P, HG, KCH, P), f32, kind="ExternalInput")
    wkbv = nc.dram_tensor("wkbv", (2, P, KCH, TQ), f32, kind="ExternalInput")
    wo = nc.dram_tensor("wo", (NHID, 2, P, 4, TQ), f32, kind="ExternalInput")
    qw = nc.dram_tensor("qw", (P, QCH), f32, kind="ExternalInput")
    kvw = nc.dram_tensor("kvw", (P, KCH), f32, kind="ExternalInput")
    cos4 = nc.dram_tensor("cos4", (P, T), f32, kind="ExternalInput")
    sin4 = nc.dram_tensor("sin4", (P, T), f32, kind="ExternalInput")  # +-sin
    cmask = nc.dram_tensor("cmask", (P, 7 * P), f32, kind="ExternalInput")
    kbias = nc.dram_tensor("kbias", (P, TC), f32, kind="ExternalInput")
    onesd = nc.dram_tensor("onesd", (P, P), f32, kind="ExternalInput")
    out = nc.dram_tensor("out", (NT, NHID, 4, P, TQ), f32, kind="ExternalOutput")

    def r(ap):
        return ap.bitcast(f32r)

    with tile.TileContext(nc) as tc, contextlib.ExitStack() as rstack:
        with (
            tc.tile_pool(name="const", bufs=1) as const,
            tc.tile_pool(name="psmm", bufs=3, space="PSUM") as psum,
            tc.tile_pool(name="pspd", bufs=2, space="PSUM") as pspd,
            tc.tile_pool(name="pssc", bufs=3, space="PSUM") as pssc,
        ):
            # ---- constants (persistent; DMAs deferred past the first
            # stage-A tiles so they don't delay the first matmuls) ----
            ones_sb = const.tile([P, P], f32r, tag="ones")
            qw_sb = const.tile([P, QCH], f32, tag="qw")
            kvw_sb = const.tile([P, KCH], f32, tag="kvw")
            cos_sb = const.tile([P, T], f32, tag="cos")
            sin_sb = const.tile([P, T], f32, tag="sin")
            kb_sb = const.tile([P, TC], f32, tag="kb")
            zero_b = const.tile([P, 1], f32, tag="zb")
            nc.vector.memset(zero_b[:], 0.0)
            eps_b = const.tile([P, 1], f32, tag="eb")
            nc.vector.memset(eps_b[:], EPS)

            def emit_const_dmas():
                nc.sync.dma_start(ones_sb[:], onesd[:, :].bitcast(f32r))
                nc.sync.dma_start(qw_sb[:], qw[:, :])
                nc.sync.dma_start(kvw_sb[:], kvw[:, :])
                nc.sync.dma_start(cos_sb[:], cos4[:, :])
                nc.sync.dma_start(sin_sb[:], sin4[:, :])
                nc.sync.dma_start(kb_sb[:], kbias[:, :])

            def rmsnorm(lat, nch, w_sb, fan, sspool, sstag, wrk):
                for t in range(NT):
                    ssp = sspool.tile([P, TQ], f32, tag=sstag, name="ssp")
                    for m in range(nch):
                        sq = wrk.tile([P, TQ], f32r, tag="sq", name="sq")
                        nc.scalar.activation(
                            sq[:],
                            lat[m][:, t * TQ : (t + 1) * TQ],
                            AF.Square,
                            bias=zero_b[:],
                        )
                        nc.tensor.matmul(
                            ssp[:],
                            r(ones_sb[:]),
                            r(sq[:]),
                            start=(m == 0),
                            stop=(m == nch - 1),
                        )
                    std = wrk.tile([P, TQ], f32, tag="std", name="std")
                    nc.scalar.activation(
                        std[:], ssp[:], AF.Sqrt, bias=eps_b[:], scale=1.0 / fan
                    )
                    rstd = wrk.tile([P, TQ], f32, tag="rstd", name="rstd")
                    with nc.allow_low_precision("rmsnorm rstd"):
                        nc.vector.reciprocal(rstd[:], std[:])
                    for m in range(nch):
                        sl = lat[m][:, t * TQ : (t + 1) * TQ]
                        nc.vector.tensor_tensor(sl, sl, rstd[:], ALU.mult)
                        nc.vector.tensor_scalar_mul(sl, sl, w_sb[:, m : m + 1])

            with tc.tile_pool(name="wqb0p", bufs=1) as wqb0p, \
                    tc.tile_pool(name="qlatp", bufs=1) as qlatp:
                wqb0 = wqb0p.tile([P, QCH, P], f32r, tag="wqb0")
                qlat = [
                    qlatp.tile([P, T], f32r, tag=f"qlat{i}", name=f"qlat{i}")
                    for i in range(QCH)
                ]
                with tc.tile_pool(name="kvlatp", bufs=1) as kvlatp:
                    kvlat = [
                        kvlatp.tile([P, T], f32r, tag=f"kvlat{i}", name=f"kvlat{i}")
                        for i in range(KCH + 1)
                    ]
                    with (
                        tc.tile_pool(name="wkbnp", bufs=1) as wkbnp,
                        tc.tile_pool(name="wkbvp", bufs=1) as wkbvp,
                    ):
                        # stage-D weights: pool reserved up front (no
                        # overlap deps); DMAs emitted mid-stage-A so they
                        # neither delay the first x tiles nor stall D
                        wkn = wkbnp.tile(
                            [P, HG, KCH, P], f32r, tag="wkbn", name="wkbn"
                        )
                        wkvts = [
                            wkbvp.tile(
                                [P, KCH, TQ], f32r, tag=f"wkbv{quad}", name="wkbv"
                            )
                            for quad in range(2)
                        ]

                        def emit_dweight_dmas():
                            nc.sync.dma_start(
                                wkn[:], wkbn[:, :, :, :].bitcast(f32r))
                            for quad in range(2):
                                nc.sync.dma_start(
                                    wkvts[quad][:],
                                    wkbv[quad, :, :, :].bitcast(f32r),
                                )
                            nc.sync.dma_start(
                                wqb0[:], wqbn[0, :, :, :].bitcast(f32r))

                        # ---- stage A: latent projections ----
                        # kv blocks first so the kv path unblocks early
                        # kpe2 lives to the end (right side)
                        kpep = rstack.enter_context(
                            tc.tile_pool(name="kpep", bufs=1, side="right"))
                        kpe2 = kpep.tile([P, T], f32r, tag="kpe2")

                        kvblocks = [
                            (wkva, 0, 256, kvlat, 0),
                            (wkva, 1, 256, kvlat, 2),
                            (wkvr, None, 64, kvlat, 4),
                        ]
                        qblocks = [
                            (wqa, mb, 256, qlat, 2 * mb)
                            for mb in range(QCH // 2)
                        ]

                        wrkA_cm = tc.tile_pool(name="wrkA", bufs=3)
                        wrkA = wrkA_cm.__enter__()
                        with (
                            tc.tile_pool(name="xk", bufs=10) as xkp,
                            tc.tile_pool(name="wA", bufs=3) as wAp,
                        ):
                            for quart in range(4):
                                xk = [None] * KQ
                                mblocks = kvblocks + qblocks
                                for mbi, (wdram, blki, width, dest, dch) in \
                                        enumerate(mblocks):
                                    nm = (width + P - 1) // P
                                    wt = wAp.tile(
                                        [P, KQ, 256], f32r, tag="wA", name="wA"
                                    )
                                    wsrc = (
                                        wdram[quart, :, :, :]
                                        if blki is None
                                        else wdram[blki, quart, :, :, :]
                                    )
                                    first = quart == 0 and mbi == 0
                                    if mbi == 0:
                                        # interleave x and weight loads
                                        for kk in range(KQ):
                                            k = quart * KQ + kk
                                            xt_ = xkp.tile(
                                                [P, T], f32r, tag="xk", name="xk"
                                            )
                                            nc.sync.dma_start(
                                                xt_[:],
                                                xT[k * P : (k + 1) * P, :].bitcast(
                                                    f32r
                                                ),
                                            )
                                            xk[kk] = xt_
                                            if first:
                                                # fine-grained first weight
                                                # loads so the kernel starts
                                                # computing immediately
                                                nc.sync.dma_start(
                                                    wt[:, kk, :width],
                                                    wsrc[:, kk, :].bitcast(f32r),
                                                )
                                                if kk == 0:
                                                    emit_const_dmas()
                                    if not first:
                                        nc.sync.dma_start(
                                            wt[:, :, :width], wsrc.bitcast(f32r)
                                        )
                                    pst = [
                                        [
                                            psum.tile(
                                                [P, TQ], f32, tag="mm", name="psA"
                                            )
                                            for _ in range(NT)
                                        ]
                                        for _ in range(nm)
                                    ]
                                    for m in range(nm):
                                        w = min(width - m * P, P)
                                        for kk in range(KQ):
                                            for t in range(NT):
                                                nc.tensor.matmul(
                                                    pst[m][t][:w, :],
                                                    r(wt[:, kk, m * P : m * P + w]),
                                                    r(
                                                        xk[kk][
                                                            :,
                                                            t * TQ : (t + 1) * TQ,
                                                        ]
                                                    ),
                                                    start=(kk == 0),
                                                    stop=(kk == KQ - 1),
                                                )
                                        for t in range(NT):
                                            dst = dest[dch + m][
                                                :w, t * TQ : (t + 1) * TQ
                                            ]
                                            if quart == 0:
                                                nc.scalar.copy(dst, pst[m][t][:w, :])
                                            else:
                                                nc.vector.tensor_tensor(
                                                    dst,
                                                    pst[m][t][:w, :],
                                                    dst,
                                                    ALU.add,
                                                )
                                    if quart == 1 and mbi == 0:
                                        emit_dweight_dmas()
                                    if quart == 3 and mbi == len(kvblocks) - 1:
                                        # kv latents final: norm + k rope now
                                        # so the kv path completes during the
                                        # remaining q blocks
                                        STAGE_MARKS.append(("A2kv", nc.next_id()))
                                        rmsnorm(kvlat, KCH, kvw_sb, KV_RANK,
                                                pspd, "pd", wrkA)
                                        ksw = xkp.tile(
                                            [P, T], f32r, tag="xk", name="ksw"
                                        )
                                        kswf = ksw[:].bitcast(f32)
                                        nc.sync.dma_start(
                                            kpe2[0:64, :], kvlat[KCH][0:64, :])
                                        nc.sync.dma_start(
                                            kpe2[64:128, :], kvlat[KCH][0:64, :])
                                        nc.sync.dma_start(
                                            kswf[0:32, :],
                                            kvlat[KCH][32:64, :].bitcast(f32))
                                        nc.sync.dma_start(
                                            kswf[32:64, :],
                                            kvlat[KCH][0:32, :].bitcast(f32))
                                        nc.sync.dma_start(
                                            kswf[64:96, :],
                                            kvlat[KCH][32:64, :].bitcast(f32))
                                        nc.sync.dma_start(
                                            kswf[96:128, :],
                                            kvlat[KCH][0:32, :].bitcast(f32))
                                        nc.vector.tensor_tensor(
                                            kswf, kswf, sin_sb[:], ALU.mult)
                                        nc.vector.tensor_tensor(
                                            kpe2[:], kpe2[:], cos_sb[:], ALU.mult)
                                        nc.vector.tensor_tensor(
                                            kpe2[:], kpe2[:], kswf, ALU.add)


                        kTnp = rstack.enter_context(
                            tc.tile_pool(name="kTnp", bufs=1, side="right"))
                        kTn = kTnp.tile([P, HG, T], f32r, tag="kTn")
                        vqp = rstack.enter_context(
                            tc.tile_pool(name="vqp", bufs=2, side="right"))
                        vq = [
                            vqp.tile([P, TC, 4 * D_V], f32r, tag="vq", name="vq")
                            for _ in range(2)
                        ]

                        STAGE_MARKS.append(("D", nc.next_id()))
                        # ---- stage D: kT_nope per head, v per quad ----
                        for h in range(HG):
                            pst = [
                                psum.tile([P, TQ], f32, tag="mm", name="psD")
                                for _ in range(NT)
                            ]
                            for k in range(KCH):
                                for t in range(NT):
                                    nc.tensor.matmul(
                                        pst[t][:],
                                        r(wkn[:, h, k, :]),
                                        r(kvlat[k][:, t * TQ : (t + 1) * TQ]),
                                        start=(k == 0),
                                        stop=(k == KCH - 1),
                                    )
                            for t in range(NT):
                                nc.vector.tensor_copy(
                                    kTn[:, h, t * TQ : (t + 1) * TQ], pst[t][:]
                                )
                        for quad in range(2):
                            for tkc in range(TC):
                                ps_ = psum.tile([P, TQ], f32, tag="mm", name="psV")
                                for k in range(KCH):
                                    nc.tensor.matmul(
                                        ps_[:],
                                        r(kvlat[k][:, tkc * P : (tkc + 1) * P]),
                                        r(wkvts[quad][:, k, :]),
                                        start=(k == 0),
                                        stop=(k == KCH - 1),
                                    )
                                nc.vector.tensor_copy(vq[quad][:, tkc, :], ps_[:])

                        STAGE_MARKS.append(("A2q", nc.next_id()))
                        rmsnorm(qlat, QCH, qw_sb, Q_RANK, pssc, "psc", wrkA)
                        wrkA_cm.__exit__(None, None, None)

                # kvlat + stage-D weight pools closed here
                actq = rstack.enter_context(
                    tc.tile_pool(name="actq", bufs=1, side="right"))
                qTn = actq.tile([P, HG, T], f32r, tag="qTn")
                qTr = actq.tile([P, HG // 2, T], f32r, tag="qTr")

                STAGE_MARKS.append(("B", nc.next_id()))
                # ---- stage B: qT per head (one DMA per head/pair) ----
                with (
                    tc.tile_pool(name="wqbp", bufs=2) as wqbp,
                    tc.tile_pool(name="qswp", bufs=1) as qswp,
                ):
                    for h in range(HG):
                        if h == 0:
                            wt = wqb0
                        else:
                            wt = wqbp.tile(
                                [P, QCH, P], f32r, tag="wqb", name="wqbn")
                            nc.sync.dma_start(
                                wt[:], wqbn[h, :, :, :].bitcast(f32r))
                        pst = [
                            psum.tile([P, TQ], f32, tag="mm", name="psB")
                            for _ in range(NT)
                        ]
                        for k in range(QCH):
                            for t in range(NT):
                                nc.tensor.matmul(
                                    pst[t][:],
                                    r(wt[:, k, :]),
                                    r(qlat[k][:, t * TQ : (t + 1) * TQ]),
                                    start=(k == 0),
                                    stop=(k == QCH - 1),
                                )
                        for t in range(NT):
                            nc.vector.tensor_copy(
                                qTn[:, h, t * TQ : (t + 1) * TQ], pst[t][:]
                            )
                    for p_ in range(HG // 2):
                        wt = wqbp.tile([P, QCH, P], f32r, tag="wqb", name="wqbr")
                        nc.sync.dma_start(wt[:], wqbr[p_, :, :, :].bitcast(f32r))
                        pst = [
                            psum.tile([P, TQ], f32, tag="mm", name="psB2")
                            for _ in range(NT)
                        ]
                        for k in range(QCH):
                            for t in range(NT):
                                nc.tensor.matmul(
                                    pst[t][:],
                                    r(wt[:, k, :]),
                                    r(qlat[k][:, t * TQ : (t + 1) * TQ]),
                                    start=(k == 0),
                                    stop=(k == QCH - 1),
                                )
                        for t in range(NT):
                            nc.vector.tensor_copy(
                                qTr[:, p_, t * TQ : (t + 1) * TQ], pst[t][:]
                            )
                        # rope this pair immediately (overlaps next pair)
                        qsw = qswp.tile([P, T], f32, tag="qsw", name="qsw")
                        qp = qTr[:, p_, :]
                        nc.sync.dma_start(qsw[0:32, :], qp[32:64, :].bitcast(f32))
                        nc.sync.dma_start(qsw[32:64, :], qp[0:32, :].bitcast(f32))
                        nc.sync.dma_start(qsw[64:96, :], qp[96:128, :].bitcast(f32))
                        nc.sync.dma_start(qsw[96:128, :], qp[64:96, :].bitcast(f32))
                        nc.vector.tensor_tensor(qsw[:], qsw[:], sin_sb[:], ALU.mult)
                        nc.vector.tensor_tensor(qp, qp, cos_sb[:], ALU.mult)
                        nc.vector.tensor_tensor(qp, qp, qsw[:], ALU.add)

            STAGE_MARKS.append(("EF", nc.next_id()))
            # kvlat + qlat pools closed here
            # ---- stages E+F per token tile (t=1 first: its leading tk
            # chunks need no causal mask, hiding the mask DMA) ----
            with (
                tc.tile_pool(name="cmp", bufs=1) as cmp_,
                tc.tile_pool(name="attp", bufs=1) as attp,
                tc.tile_pool(name="wrkE", bufs=2) as wrkE,
                tc.tile_pool(name="wop", bufs=3) as wop,
                tc.tile_pool(name="outs", bufs=2) as outp,
            ):
                # sliding causal mask: cm[dk, u] = 0 iff dk <= u - 384;
                # chunk variant rv uses columns [(3-rv)*128, (3-rv)*128+512)
                cm_sb = cmp_.tile([P, 7 * P], f32, tag="cm")
                nc.sync.dma_start(cm_sb[:], cmask[:, :])
                for t in (1, 0):
                    attnT = attp.tile([P, HG, TQ], f32r, tag="attnT", name="attnT")
                    nchunks = 4 * (t + 1)
                    for h in range(HG):
                        hb = 64 * (h % 2)
                        pd = pspd.tile([P, TQ], f32, tag="pd", name="pd")
                        pav = psum.tile([P, TQ], f32, tag="mm", name="pav")
                        for tkc in range(nchunks):
                            ps_ = pssc.tile([P, TQ], f32, tag="psc", name="psc")
                            nc.tensor.matmul(
                                ps_[:],
                                r(kTn[:, h, tkc * P : (tkc + 1) * P]),
                                r(qTn[:, h, t * TQ : (t + 1) * TQ]),
                                start=True,
                                stop=False,
                            )
                            nc.tensor.matmul(
                                ps_[:],
                                r(kpe2[hb : hb + 64, tkc * P : (tkc + 1) * P]),
                                r(
                                    qTr[
                                        hb : hb + 64,
                                        h // 2,
                                        t * TQ : (t + 1) * TQ,
                                    ]
                                ),
                                start=False,
                                stop=True,
                            )
                            if tkc >= 4 * t:
                                rv = tkc - 4 * t
                                nc.vector.tensor_tensor(
                                    ps_[:],
                                    ps_[:],
                                    cm_sb[:, (3 - rv) * P : (3 - rv) * P + TQ],
                                    ALU.add,
                                )
                            ex = wrkE.tile([P, TQ], f32r, tag="exp", name="ex")
                            nc.scalar.activation(
                                ex[:],
                                ps_[:],
                                AF.Exp,
                                bias=kb_sb[:, tkc : tkc + 1],
                                scale=SCALE,
                            )
                            nc.tensor.matmul(
                                pd[:],
                                r(ones_sb[:]),
                                r(ex[:]),
                                start=(tkc == 0),
                                stop=(tkc == nchunks - 1),
                            )
                            nc.tensor.matmul(
                                pav[:],
                                r(
                                    vq[h // 4][
                                        :, tkc, (h % 4) * P : (h % 4 + 1) * P
                                    ]
                                ),
                                r(ex[:]),
                                start=(tkc == 0),
                                stop=(tkc == nchunks - 1),
                            )
                        rec = wrkE.tile([P, TQ], f32, tag="rec", name="rec")
                        with nc.allow_low_precision("softmax denom"):
                            nc.vector.reciprocal(rec[:], pd[:])
                        nc.vector.tensor_tensor(
                            attnT[:, h, :], pav[:], rec[:], ALU.mult
                        )
                    # output projection for this token tile
                    for nt in range(NHID):
                        wts = []
                        for half in range(2):
                            wt = wop.tile([P, 4, TQ], f32r, tag="wo", name="wo")
                            nc.sync.dma_start(
                                wt[:], wo[nt, half, :, :, :].bitcast(f32r)
                            )
                            wts.append(wt)
                        for tqc in range(TQ // P):
                            po = psum.tile([P, TQ], f32, tag="mm", name="po")
                            for h in range(HG):
                                nc.tensor.matmul(
                                    po[:],
                                    r(attnT[:, h, tqc * P : (tqc + 1) * P]),
                                    r(wts[h // 4][:, h % 4, :]),
                                    start=(h == 0),
                                    stop=(h == HG - 1),
                                )
                            ot = outp.tile([P, TQ], f32, tag="osb", name="ot")
                            nc.vector.tensor_copy(ot[:], po[:])
                            nc.sync.dma_start(out[t, nt, tqc, :, :], ot[:])

    nc.finalize()
    return nc


def _get_program():
    if "nc" not in _CACHED:
        _CACHED["nc"] = _build_program()
    return _CACHED["nc"]


def _host_prep(x, wq_a, q_norm_w, wq_b, wkv_a, kv_norm_w, wkv_b, wo,
               attention_mask, positions):
    """Build the 8 per-core input maps.

    All weight tensors are host-packed partition-major so every device
    DMA is one large contiguous transfer.
    """
    f = np.float32
    x = np.asarray(x, f)
    wq_a = np.asarray(wq_a, f)
    wkv_a = np.asarray(wkv_a, f)
    wq_b3 = np.asarray(wq_b, f).reshape(Q_RANK, H, D_QK)
    wkv_b3 = np.asarray(wkv_b, f).reshape(KV_RANK, H, D_NOPE + D_V)
    wo2 = np.asarray(wo, f)
    q_norm_w = np.asarray(q_norm_w, f)
    kv_norm_w = np.asarray(kv_norm_w, f)
    attention_mask = np.asarray(attention_mask)
    positions = np.asarray(positions)

    qw = np.ascontiguousarray(q_norm_w.reshape(QCH, P).T)
    kvw = np.ascontiguousarray(kv_norm_w.reshape(KCH, P).T)

    # stage-A weights: [block, quart, p, kk, width] (replicated on all cores)
    wqa_blk = np.ascontiguousarray(
        wq_a.reshape(4, KQ, P, 6, 256).transpose(3, 0, 2, 1, 4))
    wkva_blk = np.ascontiguousarray(
        wkv_a[:, :KV_RANK].reshape(4, KQ, P, 2, 256).transpose(3, 0, 2, 1, 4))
    wkvr_blk = np.ascontiguousarray(
        wkv_a[:, KV_RANK:].reshape(4, KQ, P, 64).transpose(0, 2, 1, 3))

    inv_freq = 1.0 / (THETA ** (np.arange(0, D_ROPE, 2, dtype=np.float64) / D_ROPE))

    # sliding causal mask: cm[dk, u] = 0 iff dk <= u - 384
    dk = np.arange(P)[:, None]
    u = np.arange(7 * P)[None, :]
    cmask = np.where(dk <= u - 3 * P, 0.0, NMASK).astype(f)
    onesd = np.ones((P, P), f)

    per_batch = {}
    for b in range(B):
        xTb = np.ascontiguousarray(x[b].T)
        ang = positions[b].astype(np.float64)[:, None] * inv_freq[None, :]
        cosT = np.cos(ang).astype(f).T  # [32, T]
        sinT = np.sin(ang).astype(f).T
        cos4 = np.ascontiguousarray(np.tile(cosT, (4, 1)))
        sin4 = np.ascontiguousarray(
            np.concatenate([-sinT, sinT, -sinT, sinT], axis=0))
        kb = np.where(attention_mask[b] != 0, 0.0, NMASK).astype(f)
        kbias = np.ascontiguousarray(kb.reshape(TC, P).T)
        per_batch[b] = (xTb, cos4, sin4, kbias)

    in_maps = []
    for c in range(8):
        b, g = c // 4, c % 4
        hs = slice(g * HG, (g + 1) * HG)
        xTb, cos4, sin4, kbias = per_batch[b]
        # [rank, head, dim] -> [h, p, k, c] / [pair, p, k, c]
        wqbn_ = np.ascontiguousarray(
            wq_b3[:, hs, :D_NOPE]
            .reshape(QCH, P, HG, P).transpose(2, 1, 0, 3))
        # rope cols packed in head pairs: [h_even 64 | h_odd 64] per 128-col
        wqbr_ = np.ascontiguousarray(
            wq_b3[:, hs, D_NOPE:]
            .reshape(QCH, P, HG // 2, P).transpose(2, 1, 0, 3))
        wkbn_ = np.ascontiguousarray(
            wkv_b3[:, hs, :D_NOPE]
            .reshape(KCH, P, HG, P).transpose(1, 2, 0, 3))
        wkbv_ = np.ascontiguousarray(
            wkv_b3[:, hs, D_NOPE:]
            .reshape(KCH, P, 2, TQ).transpose(2, 1, 0, 3))
        # wo rows h*128+dv, cols nt*512+c -> [nt, half, dv(p), h%4, c]
        wosh = np.ascontiguousarray(
            wo2[g * HG * D_V : (g + 1) * HG * D_V, :]
            .reshape(2, 4, P, NHID, TQ).transpose(3, 0, 2, 1, 4))
        in_maps.append({
            "xT": xTb, "wqa": wqa_blk, "wkva": wkva_blk, "wkvr": wkvr_blk,
            "wqbn": wqbn_, "wqbr": wqbr_, "wkbn": wkbn_, "wkbv": wkbv_,
            "wo": wosh, "qw": qw, "kvw": kvw,
            "cos4": cos4, "sin4": sin4,
            "cmask": cmask, "kbias": kbias, "onesd": onesd,
        })
    return in_maps


def kernel(**inputs):
    from concourse.bass_utils import run_bass_kernel_spmd

    nc = _get_program()
    in_maps = _host_prep(**inputs)
    res = run_bass_kernel_spmd(nc, in_maps, core_ids=list(range(8)))
    _CACHED["last_result"] = res
    out = np.zeros((B, T, HID), np.float32)
    for c in range(8):
        blk = res.results[c]["out"]  # [NT, NHID, 4, P, TQ]
        # row = t*512 + q*128 + p, col = nt*512 + c
        out[c // 4] += blk.transpose(0, 2, 3, 1, 4).reshape(T, HID)
    return out


# revision 22
# speedup vs baseline: 1.2943x; 1.0396x over previous
"""DeepseekV3 MLA forward on 8 TRN2 NeuronCores.

Sharding: data-parallel over batch (B=2 -> 2 groups of 4 cores), tensor-
parallel over heads within each batch group (32 heads -> 4 groups of 8).
Each core computes its batch element's full latent projections (wq_a /
wkv_a replicated), its 8 heads' q/k/v expansions + attention, and a
partial output projection (wo row-shard); the host sums the 4 partial
outputs per batch element.

Dataflow on device keeps activations transposed ([feature, token]) so
every matmul contracts over the partition dim with no on-device
transposes anywhere:
  latentT = wq_a.T @ xT        (lhsT=wq_a cols, rhs=xT)        [rank, T]
  qT_h    = wq_b_h.T @ latentT                                  [d, T]
  scoresT = kT_h-chunks @ qT_h                                  [tk, tq]
  softmax over tk (=partitions): exp on ACT, denominator via a
  ones[128,128] matmul (result replicated across partitions), then
  attn_outT = v_chunks.T @ expT                                 [dv, tq]
  out      = attnT-chunks.T @ wo_h  (natural layout)            [tq, hid]
RoPE in transposed layout: rot(x) = x*cos + swap32(x)*(+-sin), where
swap32 exchanges the two 32-row halves of each 64-row rope block (done
with SBUF->SBUF block DMAs) and the +-sin sign pattern is host-built.
All matmuls run as float32r (FP22-truncated fp32, single pass).

Perf notes: per-DMA issue costs ~1us on the sync sequencer, so weights
are host-packed into large partition-major blocks and DMA'd in few big
transfers (SBUF slices feed the matmuls). SBUF pools are statically
reserved; they are scoped per stage, long-lived activation pools on the
"right" allocator side. PSUM: 8 banks split mm:4 / pd:2 / sc:2.
"""

import os
import sys

import numpy as np

sys.path.insert(0, "/opt/trn_rl_repo")

B, T, HID = 2, 1024, 4096
H, D_NOPE, D_ROPE, D_V = 32, 128, 64, 128
D_QK = D_NOPE + D_ROPE
Q_RANK, KV_RANK = 1536, 512
THETA, EPS = 10000.0, 1e-6
SCALE = float(D_QK) ** -0.5
NMASK = -30000.0

HG = H // 4          # heads per core = 8
P = 128
QCH = Q_RANK // P    # 12 latent chunks (q)
KCH = KV_RANK // P   # 4 latent chunks (kv)
HIDK = HID // P      # 32 contraction tiles for stage A
KQ = HIDK // 4       # 8 k-tiles per stage-A quarter
TQ = 512             # token tile (free dim) for most matmuls
NT = T // TQ         # 2 token tiles
TC = T // P          # 8 token chunks of 128
NHID = HID // TQ     # 8 output column tiles

_CACHED = {}
STAGE_MARKS = []


def _build_program():
    import contextlib

    import concourse.bacc as bacc
    import concourse.mybir as mybir
    import concourse.tile as tile

    f32 = mybir.dt.float32
    f32r = mybir.dt.float32r
    AF = mybir.ActivationFunctionType
    ALU = mybir.AluOpType

    nc = bacc.Bacc()

    # ---- DRAM I/O (per-core shapes; SPMD across the 8 cores) ----
    # weights are host-packed partition-major so each DMA is one big
    # contiguous transfer whose SBUF image is sliced per matmul
    xT = nc.dram_tensor("xT", (HID, T), f32, kind="ExternalInput")
    wqa = nc.dram_tensor("wqa", (6, 4, P, KQ, 256), f32, kind="ExternalInput")
    wkva = nc.dram_tensor("wkva", (2, 4, P, KQ, 256), f32, kind="ExternalInput")
    wkvr = nc.dram_tensor("wkvr", (4, P, KQ, 64), f32, kind="ExternalInput")
    wqbn = nc.dram_tensor("wqbn", (HG, P, QCH, P), f32, kind="ExternalInput")
    wqbr = nc.dram_tensor("wqbr", (HG // 2, P, QCH, P), f32, kind="ExternalInput")
    wkbn = nc.dram_tensor("wkbn", (P, HG, KCH, P), f32, kind="ExternalInput")
    wkbv = nc.dram_tensor("wkbv", (2, P, KCH, TQ), f32, kind="ExternalInput")
    wo = nc.dram_tensor("wo", (NHID, 2, P, 4, TQ), f32, kind="ExternalInput")
    qw = nc.dram_tensor("qw", (P, QCH), f32, kind="ExternalInput")
    kvw = nc.dram_tensor("kvw", (P, KCH), f32, kind="ExternalInput")
    cos4 = nc.dram_tensor("cos4", (P, T), f32, kind="ExternalInput")
    sin4 = nc.dram_tensor("sin4", (P, T), f32, kind="ExternalInput")  # +-sin
    cmask = nc.dram_tensor("cmask", (P, 7 * P), f32, kind="ExternalInput")
    kbias = nc.dram_tensor("kbias", (P, TC), f32, kind="ExternalInput")
    onesd = nc.dram_tensor("onesd", (P, P), f32, kind="ExternalInput")
    out = nc.dram_tensor("out", (NT, NHID, 4, P, TQ), f32, kind="ExternalOutput")

    def r(ap):
        return ap.bitcast(f32r)

    with tile.TileContext(nc) as tc, contextlib.ExitStack() as rstack:
        with (
            tc.tile_pool(name="const", bufs=1) as const,
            tc.tile_pool(name="psmm", bufs=3, space="PSUM") as psum,
            tc.tile_pool(name="pspd", bufs=2, space="PSUM") as pspd,
            tc.tile_pool(name="pssc", bufs=3, space="PSUM") as pssc,
        ):
            # ---- constants (persistent; DMAs deferred past the first
            # stage-A tiles so they don't delay the first matmuls) ----
            ones_sb = const.tile([P, P], f32r, tag="ones")
            qw_sb = const.tile([P, QCH], f32, tag="qw")
            kvw_sb = const.tile([P, KCH], f32, tag="kvw")
            cos_sb = const.tile([P, T], f32, tag="cos")
            sin_sb = const.tile([P, T], f32, tag="sin")
            kb_sb = const.tile([P, TC], f32, tag="kb")
            zero_b = const.tile([P, 1], f32, tag="zb")
            nc.vector.memset(zero_b[:], 0.0)
            eps_b = const.tile([P, 1], f32, tag="eb")
            nc.vector.memset(eps_b[:], EPS)

            def emit_const_dmas():
                nc.sync.dma_start(ones_sb[:], onesd[:, :].bitcast(f32r))
                nc.sync.dma_start(qw_sb[:], qw[:, :])
                nc.sync.dma_start(kvw_sb[:], kvw[:, :])
                nc.sync.dma_start(cos_sb[:], cos4[:, :])
                nc.sync.dma_start(sin_sb[:], sin4[:, :])
                nc.sync.dma_start(kb_sb[:], kbias[:, :])

            def rmsnorm(lat, nch, w_sb, fan, sspool, sstag, wrk):
                for t in range(NT):
                    ssp = sspool.tile([P, TQ], f32, tag=sstag, name="ssp")
                    for m in range(nch):
                        sq = wrk.tile([P, TQ], f32r, tag="sq", name="sq")
                        nc.scalar.activation(
                            sq[:],
                            lat[m][:, t * TQ : (t + 1) * TQ],
                            AF.Square,
                            bias=zero_b[:],
                        )
                        nc.tensor.matmul(
                            ssp[:],
                            r(ones_sb[:]),
                            r(sq[:]),
                            start=(m == 0),
                            stop=(m == nch - 1),
                        )
                    std = wrk.tile([P, TQ], f32, tag="std", name="std")
                    nc.scalar.activation(
                        std[:], ssp[:], AF.Sqrt, bias=eps_b[:], scale=1.0 / fan
                    )
                    rstd = wrk.tile([P, TQ], f32, tag="rstd", name="rstd")
                    with nc.allow_low_precision("rmsnorm rstd"):
                        nc.vector.reciprocal(rstd[:], std[:])
                    for m in range(nch):
                        sl = lat[m][:, t * TQ : (t + 1) * TQ]
                        nc.vector.tensor_tensor(sl, sl, rstd[:], ALU.mult)
                        nc.vector.tensor_scalar_mul(sl, sl, w_sb[:, m : m + 1])

            with tc.tile_pool(name="wqb0p", bufs=1) as wqb0p, \
                    tc.tile_pool(name="qlatp", bufs=1) as qlatp:
                wqb0 = wqb0p.tile([P, QCH, P], f32r, tag="wqb0")
                qlat = [
                    qlatp.tile([P, T], f32r, tag=f"qlat{i}", name=f"qlat{i}")
                    for i in range(QCH)
                ]
                with tc.tile_pool(name="kvlatp", bufs=1) as kvlatp:
                    kvlat = [
                        kvlatp.tile([P, T], f32r, tag=f"kvlat{i}", name=f"kvlat{i}")
                        for i in range(KCH + 1)
                    ]
                    with (
                        tc.tile_pool(name="wkbnp", bufs=1) as wkbnp,
                        tc.tile_pool(name="wkbvp", bufs=1) as wkbvp,
                    ):
                        # stage-D weights: pool reserved up front (no
                        # overlap deps); DMAs emitted mid-stage-A so they
                        # neither delay the first x tiles nor stall D
                        wkn = wkbnp.tile(
                            [P, HG, KCH, P], f32r, tag="wkbn", name="wkbn"
                        )
                        wkvts = [
                            wkbvp.tile(
                                [P, KCH, TQ], f32r, tag=f"wkbv{quad}", name="wkbv"
                            )
                            for quad in range(2)
                        ]

                        def emit_dweight_dmas():
                            nc.sync.dma_start(
                                wkn[:], wkbn[:, :, :, :].bitcast(f32r))
                            for quad in range(2):
                                nc.sync.dma_start(
                                    wkvts[quad][:],
                                    wkbv[quad, :, :, :].bitcast(f32r),
                                )
                            nc.sync.dma_start(
                                wqb0[:], wqbn[0, :, :, :].bitcast(f32r))

                        # ---- stage A: latent projections ----
                        # kv blocks first so the kv path unblocks early
                        # kpe2 lives to the end (right side)
                        kpep = rstack.enter_context(
                            tc.tile_pool(name="kpep", bufs=1, side="right"))
                        kpe2 = kpep.tile([P, T], f32r, tag="kpe2")

                        kvblocks = [
                            (wkva, 0, 256, kvlat, 0),
                            (wkva, 1, 256, kvlat, 2),
                            (wkvr, None, 64, kvlat, 4),
                        ]
                        qblocks = [
                            (wqa, mb, 256, qlat, 2 * mb)
                            for mb in range(QCH // 2)
                        ]

                        wrkA_cm = tc.tile_pool(name="wrkA", bufs=3)
                        wrkA = wrkA_cm.__enter__()
                        with (
                            tc.tile_pool(name="xk", bufs=10) as xkp,
                            tc.tile_pool(name="wA", bufs=3) as wAp,
                        ):
                            for quart in range(4):
                                xk = [None] * KQ
                                mblocks = kvblocks + qblocks
                                for mbi, (wdram, blki, width, dest, dch) in \
                                        enumerate(mblocks):
                                    nm = (width + P - 1) // P
                                    wt = wAp.tile(
                                        [P, KQ, 256], f32r, tag="wA", name="wA"
                                    )
                                    wsrc = (
                                        wdram[quart, :, :, :]
                                        if blki is None
                                        else wdram[blki, quart, :, :, :]
                                    )
                                    first = quart == 0 and mbi == 0
                                    if mbi == 0:
                                        # interleave x and weight loads
                                        for kk in range(KQ):
                                            k = quart * KQ + kk
                                            xt_ = xkp.tile(
                                                [P, T], f32r, tag="xk", name="xk"
                                            )
                                            nc.sync.dma_start(
                                                xt_[:],
                                                xT[k * P : (k + 1) * P, :].bitcast(
                                                    f32r
                                                ),
                                            )
                                            xk[kk] = xt_
                                            if first:
                                                # fine-grained first weight
                                                # loads so the kernel starts
                                                # computing immediately
                                                nc.sync.dma_start(
                                                    wt[:, kk, :width],
                                                    wsrc[:, kk, :].bitcast(f32r),
                                                )
                                                if kk == 0:
                                                    emit_const_dmas()
                                    if not first:
                                        nc.sync.dma_start(
                                            wt[:, :, :width], wsrc.bitcast(f32r)
                                        )
                                    pst = [
                                        [
                                            psum.tile(
                                                [P, TQ], f32, tag="mm", name="psA"
                                            )
                                            for _ in range(NT)
                                        ]
                                        for _ in range(nm)
                                    ]
                                    for m in range(nm):
                                        w = min(width - m * P, P)
                                        for kk in range(KQ):
                                            for t in range(NT):
                                                nc.tensor.matmul(
                                                    pst[m][t][:w, :],
                                                    r(wt[:, kk, m * P : m * P + w]),
                                                    r(
                                                        xk[kk][
                                                            :,
                                                            t * TQ : (t + 1) * TQ,
                                                        ]
                                                    ),
                                                    start=(kk == 0),
                                                    stop=(kk == KQ - 1),
                                                )
                                        for t in range(NT):
                                            dst = dest[dch + m][
                                                :w, t * TQ : (t + 1) * TQ
                                            ]
                                            if quart == 0:
                                                nc.scalar.copy(dst, pst[m][t][:w, :])
                                            else:
                                                nc.vector.tensor_tensor(
                                                    dst,
                                                    pst[m][t][:w, :],
                                                    dst,
                                                    ALU.add,
                                                )
                                    if quart == 1 and mbi == 0:
                                        emit_dweight_dmas()
                                    if quart == 3 and mbi == len(kvblocks) - 1:
                                        # kv latents final: norm + k rope now
                                        # so the kv path completes during the
                                        # remaining q blocks
                                        STAGE_MARKS.append(("A2kv", nc.next_id()))
                                        rmsnorm(kvlat, KCH, kvw_sb, KV_RANK,
                                                pspd, "pd", wrkA)
                                        ksw = xkp.tile(
                                            [P, T], f32r, tag="xk", name="ksw"
                                        )
                                        kswf = ksw[:].bitcast(f32)
                                        nc.sync.dma_start(
                                            kpe2[0:64, :], kvlat[KCH][0:64, :])
                                        nc.sync.dma_start(
                                            kpe2[64:128, :], kvlat[KCH][0:64, :])
                                        nc.sync.dma_start(
                                            kswf[0:32, :],
                                            kvlat[KCH][32:64, :].bitcast(f32))
                                        nc.sync.dma_start(
                                            kswf[32:64, :],
                                            kvlat[KCH][0:32, :].bitcast(f32))
                                        nc.sync.dma_start(
                                            kswf[64:96, :],
                                            kvlat[KCH][32:64, :].bitcast(f32))
                                        nc.sync.dma_start(
                                            kswf[96:128, :],
                                            kvlat[KCH][0:32, :].bitcast(f32))
                                        nc.vector.tensor_tensor(
                                            kswf, kswf, sin_sb[:], ALU.mult)
                                        nc.vector.tensor_tensor(
                                            kpe2[:], kpe2[:], cos_sb[:], ALU.mult)
                                        nc.vector.tensor_tensor(
                                            kpe2[:], kpe2[:], kswf, ALU.add)


                        kTnp = rstack.enter_context(
                            tc.tile_pool(name="kTnp", bufs=1, side="right"))
                        kTn = kTnp.tile([P, HG, T], f32r, tag="kTn")
                        vqp = rstack.enter_context(
                            tc.tile_pool(name="vqp", bufs=2, side="right"))
                        vq = [
                            vqp.tile([P, TC, 4 * D_V], f32r, tag="vq", name="vq")
                            for _ in range(2)
                        ]

                        STAGE_MARKS.append(("D", nc.next_id()))
                        # ---- stage D: kT_nope per head, v per quad ----
                        for h in range(HG):
                            pst = [
                                psum.tile([P, TQ], f32, tag="mm", name="psD")
                                for _ in range(NT)
                            ]
                            for k in range(KCH):
                                for t in range(NT):
                                    nc.tensor.matmul(
                                        pst[t][:],
                                        r(wkn[:, h, k, :]),
                                        r(kvlat[k][:, t * TQ : (t + 1) * TQ]),
                                        start=(k == 0),
                                        stop=(k == KCH - 1),
                                    )
                            for t in range(NT):
                                nc.vector.tensor_copy(
                                    kTn[:, h, t * TQ : (t + 1) * TQ], pst[t][:]
                                )
                        for quad in range(2):
                            for tkc in range(TC):
                                ps_ = psum.tile([P, TQ], f32, tag="mm", name="psV")
                                for k in range(KCH):
                                    nc.tensor.matmul(
                                        ps_[:],
                                        r(kvlat[k][:, tkc * P : (tkc + 1) * P]),
                                        r(wkvts[quad][:, k, :]),
                                        start=(k == 0),
                                        stop=(k == KCH - 1),
                                    )
                                nc.vector.tensor_copy(vq[quad][:, tkc, :], ps_[:])

                        STAGE_MARKS.append(("A2q", nc.next_id()))
                        rmsnorm(qlat, QCH, qw_sb, Q_RANK, pssc, "psc", wrkA)
                        wrkA_cm.__exit__(None, None, None)

                # kvlat + stage-D weight pools closed here
                actq = rstack.enter_context(
                    tc.tile_pool(name="actq", bufs=1, side="right"))
                qTn = actq.tile([P, HG, T], f32r, tag="qTn")
                qTr = actq.tile([P, HG // 2, T], f32r, tag="qTr")

                STAGE_MARKS.append(("B", nc.next_id()))
                # ---- stage B: qT per head (one DMA per head/pair) ----
                with (
                    tc.tile_pool(name="wqbp", bufs=2) as wqbp,
                    tc.tile_pool(name="qswp", bufs=1) as qswp,
                ):
                    for h in range(HG):
                        if h == 0:
                            wt = wqb0
                        else:
                            wt = wqbp.tile(
                                [P, QCH, P], f32r, tag="wqb", name="wqbn")
                            nc.sync.dma_start(
                                wt[:], wqbn[h, :, :, :].bitcast(f32r))
                        pst = [
                            psum.tile([P, TQ], f32, tag="mm", name="psB")
                            for _ in range(NT)
                        ]
                        for k in range(QCH):
                            for t in range(NT):
                                nc.tensor.matmul(
                                    pst[t][:],
                                    r(wt[:, k, :]),
                                    r(qlat[k][:, t * TQ : (t + 1) * TQ]),
                                    start=(k == 0),
                                    stop=(k == QCH - 1),
                                )
                        for t in range(NT):
                            nc.vector.tensor_copy(
                                qTn[:, h, t * TQ : (t + 1) * TQ], pst[t][:]
                            )
                    for p_ in range(HG // 2):
                        wt = wqbp.tile([P, QCH, P], f32r, tag="wqb", name="wqbr")
                        nc.sync.dma_start(wt[:], wqbr[p_, :, :, :].bitcast(f32r))
                        pst = [
                            psum.tile([P, TQ], f32, tag="mm", name="psB2")
                            for _ in range(NT)
                        ]
                        for k in range(QCH):
                            for t in range(NT):
                                nc.tensor.matmul(
                                    pst[t][:],
                                    r(wt[:, k, :]),
                                    r(qlat[k][:, t * TQ : (t + 1) * TQ]),
                                    start=(k == 0),
                                    stop=(k == QCH - 1),
                                )
                        for t in range(NT):
                            nc.vector.tensor_copy(
                                qTr[:, p_, t * TQ : (t + 1) * TQ], pst[t][:]
                            )
                        # rope this pair immediately (overlaps next pair)
                        qsw = qswp.tile([P, T], f32, tag="qsw", name="qsw")
                        qp = qTr[:, p_, :]
                        nc.sync.dma_start(qsw[0:32, :], qp[32:64, :].bitcast(f32))
                        nc.sync.dma_start(qsw[32:64, :], qp[0:32, :].bitcast(f32))
                        nc.sync.dma_start(qsw[64:96, :], qp[96:128, :].bitcast(f32))
                        nc.sync.dma_start(qsw[96:128, :], qp[64:96, :].bitcast(f32))
                        nc.vector.tensor_tensor(qsw[:], qsw[:], sin_sb[:], ALU.mult)
                        nc.vector.tensor_tensor(qp, qp, cos_sb[:], ALU.mult)
                        nc.vector.tensor_tensor(qp, qp, qsw[:], ALU.add)

            STAGE_MARKS.append(("EF", nc.next_id()))
            # kvlat + qlat pools closed here
            # ---- stages E+F per token tile (t=1 first: its leading tk
            # chunks need no causal mask, hiding the mask DMA) ----
            with (
                tc.tile_pool(name="cmp", bufs=1) as cmp_,
                tc.tile_pool(name="attp", bufs=1) as attp,
                tc.tile_pool(name="wrkE", bufs=3) as wrkE,
                tc.tile_pool(name="wop", bufs=3) as wop,
                tc.tile_pool(name="outs", bufs=2) as outp,
            ):
                # sliding causal mask: cm[dk, u] = 0 iff dk <= u - 384;
                # chunk variant rv uses columns [(3-rv)*128, (3-rv)*128+512)
                cm_sb = cmp_.tile([P, 7 * P], f32, tag="cm")
                nc.sync.dma_start(cm_sb[:], cmask[:, :])
                for t in (1, 0):
                    attnT = attp.tile([P, HG, TQ], f32r, tag="attnT", name="attnT")
                    nchunks = 4 * (t + 1)
                    for h in range(HG):
                        hb = 64 * (h % 2)
                        pd = pspd.tile([P, TQ], f32, tag="pd", name="pd")
                        pav = psum.tile([P, TQ], f32, tag="mm", name="pav")
                        for tkc in range(nchunks):
                            ps_ = pssc.tile([P, TQ], f32, tag="psc", name="psc")
                            nc.tensor.matmul(
                                ps_[:],
                                r(kTn[:, h, tkc * P : (tkc + 1) * P]),
                                r(qTn[:, h, t * TQ : (t + 1) * TQ]),
                                start=True,
                                stop=False,
                            )
                            nc.tensor.matmul(
                                ps_[:],
                                r(kpe2[hb : hb + 64, tkc * P : (tkc + 1) * P]),
                                r(
                                    qTr[
                                        hb : hb + 64,
                                        h // 2,
                                        t * TQ : (t + 1) * TQ,
                                    ]
                                ),
                                start=False,
                                stop=True,
                            )
                            if tkc >= 4 * t:
                                rv = tkc - 4 * t
                                nc.vector.tensor_tensor(
                                    ps_[:],
                                    ps_[:],
                                    cm_sb[:, (3 - rv) * P : (3 - rv) * P + TQ],
                                    ALU.add,
                                )
                            ex = wrkE.tile([P, TQ], f32r, tag="exp", name="ex")
                            nc.scalar.activation(
                                ex[:],
                                ps_[:],
                                AF.Exp,
                                bias=kb_sb[:, tkc : tkc + 1],
                                scale=SCALE,
                            )
                            nc.tensor.matmul(
                                pd[:],
                                r(ones_sb[:]),
                                r(ex[:]),
                                start=(tkc == 0),
                                stop=(tkc == nchunks - 1),
                            )
                            nc.tensor.matmul(
                                pav[:],
                                r(
                                    vq[h // 4][
                                        :, tkc, (h % 4) * P : (h % 4 + 1) * P
                                    ]
                                ),
                                r(ex[:]),
                                start=(tkc == 0),
                                stop=(tkc == nchunks - 1),
                            )
                        rec = wrkE.tile([P, TQ], f32, tag="rec", name="rec")
                        with nc.allow_low_precision("softmax denom"):
                            nc.vector.reciprocal(rec[:], pd[:])
                        nc.vector.tensor_tensor(
                            attnT[:, h, :], pav[:], rec[:], ALU.mult
                        )
                    # output projection for this token tile
                    for nt in range(NHID):
                        wts = []
                        for half in range(2):
                            wt = wop.tile([P, 4, TQ], f32r, tag="wo", name="wo")
                            nc.sync.dma_start(
                                wt[:], wo[nt, half, :, :, :].bitcast(f32r)
                            )
                            wts.append(wt)
                        for tqc in range(TQ // P):
                            po = psum.tile([P, TQ], f32, tag="mm", name="po")
                            for h in range(HG):
                                nc.tensor.matmul(
                                    po[:],
                                    r(attnT[:, h, tqc * P : (tqc + 1) * P]),
                                    r(wts[h // 4][:, h % 4, :]),
                                    start=(h == 0),
                                    stop=(h == HG - 1),
                                )
                            ot = outp.tile([P, TQ], f32, tag="osb", name="ot")
                            nc.vector.tensor_copy(ot[:], po[:])
                            nc.sync.dma_start(out[t, nt, tqc, :, :], ot[:])

    nc.finalize()
    return nc


def _get_program():
    if "nc" not in _CACHED:
        _CACHED["nc"] = _build_program()
    return _CACHED["nc"]


def _host_prep(x, wq_a, q_norm_w, wq_b, wkv_a, kv_norm_w, wkv_b, wo,
               attention_mask, positions):
    """Build the 8 per-core input maps.

    All weight tensors are host-packed partition-major so every device
    DMA is one large contiguous transfer.
    """
    f = np.float32
    x = np.asarray(x, f)
    wq_a = np.asarray(wq_a, f)
    wkv_a = np.asarray(wkv_a, f)
    wq_b3 = np.asarray(wq_b, f).reshape(Q_RANK, H, D_QK)
    wkv_b3 = np.asarray(wkv_b, f).reshape(KV_RANK, H, D_NOPE + D_V)
    wo2 = np.asarray(wo, f)
    q_norm_w = np.asarray(q_norm_w, f)
    kv_norm_w = np.asarray(kv_norm_w, f)
    attention_mask = np.asarray(attention_mask)
    positions = np.asarray(positions)

    qw = np.ascontiguousarray(q_norm_w.reshape(QCH, P).T)
    kvw = np.ascontiguousarray(kv_norm_w.reshape(KCH, P).T)

    # stage-A weights: [block, quart, p, kk, width] (replicated on all cores)
    wqa_blk = np.ascontiguousarray(
        wq_a.reshape(4, KQ, P, 6, 256).transpose(3, 0, 2, 1, 4))
    wkva_blk = np.ascontiguousarray(
        wkv_a[:, :KV_RANK].reshape(4, KQ, P, 2, 256).transpose(3, 0, 2, 1, 4))
    wkvr_blk = np.ascontiguousarray(
        wkv_a[:, KV_RANK:].reshape(4, KQ, P, 64).transpose(0, 2, 1, 3))

    inv_freq = 1.0 / (THETA ** (np.arange(0, D_ROPE, 2, dtype=np.float64) / D_ROPE))

    # sliding causal mask: cm[dk, u] = 0 iff dk <= u - 384
    dk = np.arange(P)[:, None]
    u = np.arange(7 * P)[None, :]
    cmask = np.where(dk <= u - 3 * P, 0.0, NMASK).astype(f)
    onesd = np.ones((P, P), f)

    per_batch = {}
    for b in range(B):
        xTb = np.ascontiguousarray(x[b].T)
        ang = positions[b].astype(np.float64)[:, None] * inv_freq[None, :]
        cosT = np.cos(ang).astype(f).T  # [32, T]
        sinT = np.sin(ang).astype(f).T
        cos4 = np.ascontiguousarray(np.tile(cosT, (4, 1)))
        sin4 = np.ascontiguousarray(
            np.concatenate([-sinT, sinT, -sinT, sinT], axis=0))
        kb = np.where(attention_mask[b] != 0, 0.0, NMASK).astype(f)
        kbias = np.ascontiguousarray(kb.reshape(TC, P).T)
        per_batch[b] = (xTb, cos4, sin4, kbias)

    in_maps = []
    for c in range(8):
        b, g = c // 4, c % 4
        hs = slice(g * HG, (g + 1) * HG)
        xTb, cos4, sin4, kbias = per_batch[b]
        # [rank, head, dim] -> [h, p, k, c] / [pair, p, k, c]
        wqbn_ = np.ascontiguousarray(
            wq_b3[:, hs, :D_NOPE]
            .reshape(QCH, P, HG, P).transpose(2, 1, 0, 3))
        # rope cols packed in head pairs: [h_even 64 | h_odd 64] per 128-col
        wqbr_ = np.ascontiguousarray(
            wq_b3[:, hs, D_NOPE:]
            .reshape(QCH, P, HG // 2, P).transpose(2, 1, 0, 3))
        wkbn_ = np.ascontiguousarray(
            wkv_b3[:, hs, :D_NOPE]
            .reshape(KCH, P, HG, P).transpose(1, 2, 0, 3))
        wkbv_ = np.ascontiguousarray(
            wkv_b3[:, hs, D_NOPE:]
            .reshape(KCH, P, 2, TQ).transpose(2, 1, 0, 3))
        # wo rows h*128+dv, cols nt*512+c -> [nt, half, dv(p), h%4, c]
        wosh = np.ascontiguousarray(
            wo2[g * HG * D_V : (g + 1) * HG * D_V, :]
            .reshape(2, 4, P, NHID, TQ).transpose(3, 0, 2, 1, 4))
        in_maps.append({
            "xT": xTb, "wqa": wqa_blk, "wkva": wkva_blk, "wkvr": wkvr_blk,
            "wqbn": wqbn_, "wqbr": wqbr_, "wkbn": wkbn_, "wkbv": wkbv_,
            "wo": wosh, "qw": qw, "kvw": kvw,
            "cos4": cos4, "sin4": sin4,
            "cmask": cmask, "kbias": kbias, "onesd": onesd,
        })
    return in_maps


def kernel(**inputs):
    from concourse.bass_utils import run_bass_kernel_spmd

    nc = _get_program()
    in_maps = _host_prep(**inputs)
    res = run_bass_kernel_spmd(nc, in_maps, core_ids=list(range(8)))
    _CACHED["last_result"] = res
    out = np.zeros((B, T, HID), np.float32)
    for c in range(8):
        blk = res.results[c]["out"]  # [NT, NHID, 4, P, TQ]
        # row = t*512 + q*128 + p, col = nt*512 + c
        out[c // 4] += blk.transpose(0, 2, 3, 1, 4).reshape(T, HID)
    return out


# revision 25
# speedup vs baseline: 1.3490x; 1.0423x over previous
"""DeepseekV3 MLA forward on 8 TRN2 NeuronCores.

Sharding: data-parallel over batch (B=2 -> 2 groups of 4 cores), tensor-
parallel over heads within each batch group (32 heads -> 4 groups of 8).
Each core computes its batch element's full latent projections (wq_a /
wkv_a replicated), its 8 heads' q/k/v expansions + attention, and a
partial output projection (wo row-shard); the host sums the 4 partial
outputs per batch element.

Dataflow on device keeps activations transposed ([feature, token]) so
every matmul contracts over the partition dim with no on-device
transposes anywhere:
  latentT = wq_a.T @ xT        (lhsT=wq_a cols, rhs=xT)        [rank, T]
  qT_h    = wq_b_h.T @ latentT                                  [d, T]
  scoresT = kT_h-chunks @ qT_h                                  [tk, tq]
  softmax over tk (=partitions): exp on ACT, denominator via a
  ones[128,128] matmul (result replicated across partitions), then
  attn_outT = v_chunks.T @ expT                                 [dv, tq]
  out      = attnT-chunks.T @ wo_h  (natural layout)            [tq, hid]
RoPE in transposed layout: rot(x) = x*cos + swap32(x)*(+-sin), where
swap32 exchanges the two 32-row halves of each 64-row rope block (done
with SBUF->SBUF block DMAs) and the +-sin sign pattern is host-built.
All matmuls run as float32r (FP22-truncated fp32, single pass).

Perf notes: per-DMA issue costs ~1us on the sync sequencer, so weights
are host-packed into large partition-major blocks and DMA'd in few big
transfers (SBUF slices feed the matmuls). SBUF pools are statically
reserved; they are scoped per stage, long-lived activation pools on the
"right" allocator side. PSUM: 8 banks split mm:4 / pd:2 / sc:2.
"""

import os
import sys

import numpy as np

sys.path.insert(0, "/opt/trn_rl_repo")

B, T, HID = 2, 1024, 4096
H, D_NOPE, D_ROPE, D_V = 32, 128, 64, 128
D_QK = D_NOPE + D_ROPE
Q_RANK, KV_RANK = 1536, 512
THETA, EPS = 10000.0, 1e-6
SCALE = float(D_QK) ** -0.5
NMASK = -30000.0

HG = H // 4          # heads per core = 8
P = 128
QCH = Q_RANK // P    # 12 latent chunks (q)
KCH = KV_RANK // P   # 4 latent chunks (kv)
HIDK = HID // P      # 32 contraction tiles for stage A
KQ = HIDK // 4       # 8 k-tiles per stage-A quarter
TQ = 512             # token tile (free dim) for most matmuls
NT = T // TQ         # 2 token tiles
TC = T // P          # 8 token chunks of 128
NHID = HID // TQ     # 8 output column tiles

_CACHED = {}
STAGE_MARKS = []


def _build_program():
    import contextlib

    import concourse.bacc as bacc
    import concourse.mybir as mybir
    import concourse.tile as tile

    f32 = mybir.dt.float32
    f32r = mybir.dt.float32r
    AF = mybir.ActivationFunctionType
    ALU = mybir.AluOpType

    nc = bacc.Bacc()

    # ---- DRAM I/O (per-core shapes; SPMD across the 8 cores) ----
    # weights are host-packed partition-major so each DMA is one big
    # contiguous transfer whose SBUF image is sliced per matmul
    xT = nc.dram_tensor("xT", (HID, T), f32, kind="ExternalInput")
    wqa = nc.dram_tensor("wqa", (6, 4, P, KQ, 256), f32, kind="ExternalInput")
    wkva = nc.dram_tensor("wkva", (2, 4, P, KQ, 256), f32, kind="ExternalInput")
    wkvr = nc.dram_tensor("wkvr", (4, P, KQ, 64), f32, kind="ExternalInput")
    wqbn = nc.dram_tensor("wqbn", (HG, P, QCH, P), f32, kind="ExternalInput")
    wqbr = nc.dram_tensor("wqbr", (HG // 2, P, QCH, P), f32, kind="ExternalInput")
    wkbn = nc.dram_tensor("wkbn", (P, HG, KCH, P), f32, kind="ExternalInput")
    wkbv = nc.dram_tensor("wkbv", (2, P, KCH, TQ), f32, kind="ExternalInput")
    wo = nc.dram_tensor("wo", (NHID, 2, P, 4, TQ), f32, kind="ExternalInput")
    qw = nc.dram_tensor("qw", (P, QCH), f32, kind="ExternalInput")
    kvw = nc.dram_tensor("kvw", (P, KCH), f32, kind="ExternalInput")
    cos4 = nc.dram_tensor("cos4", (P, T), f32, kind="ExternalInput")
    sin4 = nc.dram_tensor("sin4", (P, T), f32, kind="ExternalInput")  # +-sin
    cmask = nc.dram_tensor("cmask", (P, 7 * P), f32, kind="ExternalInput")
    kbias = nc.dram_tensor("kbias", (P, TC), f32, kind="ExternalInput")
    onesd = nc.dram_tensor("onesd", (P, P), f32, kind="ExternalInput")
    out = nc.dram_tensor("out", (NT, NHID, 4, P, TQ), f32, kind="ExternalOutput")

    def r(ap):
        return ap.bitcast(f32r)

    with tile.TileContext(nc) as tc, contextlib.ExitStack() as rstack:
        with (
            tc.tile_pool(name="const", bufs=1) as const,
            tc.tile_pool(name="psmm", bufs=3, space="PSUM") as psum,
            tc.tile_pool(name="pspd", bufs=2, space="PSUM") as pspd,
            tc.tile_pool(name="pssc", bufs=3, space="PSUM") as pssc,
        ):
            # ---- constants (persistent; DMAs deferred past the first
            # stage-A tiles so they don't delay the first matmuls) ----
            ones_sb = const.tile([P, P], f32r, tag="ones")
            qw_sb = const.tile([P, QCH], f32, tag="qw")
            kvw_sb = const.tile([P, KCH], f32, tag="kvw")
            cos_sb = const.tile([P, T], f32, tag="cos")
            sin_sb = const.tile([P, T], f32, tag="sin")
            kb_sb = const.tile([P, TC], f32, tag="kb")
            zero_b = const.tile([P, 1], f32, tag="zb")
            nc.vector.memset(zero_b[:], 0.0)
            eps_b = const.tile([P, 1], f32, tag="eb")
            nc.vector.memset(eps_b[:], EPS)

            def emit_const_dmas():
                nc.sync.dma_start(ones_sb[:], onesd[:, :].bitcast(f32r))
                nc.sync.dma_start(qw_sb[:], qw[:, :])
                nc.sync.dma_start(kvw_sb[:], kvw[:, :])
                nc.sync.dma_start(cos_sb[:], cos4[:, :])
                nc.sync.dma_start(sin_sb[:], sin4[:, :])
                nc.sync.dma_start(kb_sb[:], kbias[:, :])

            def rmsnorm(lat, nch, w_sb, fan, sspool, sstag, wrk):
                for t in range(NT):
                    ssp = sspool.tile([P, TQ], f32, tag=sstag, name="ssp")
                    for m in range(nch):
                        sq = wrk.tile([P, TQ], f32r, tag="sq", name="sq")
                        nc.scalar.activation(
                            sq[:],
                            lat[m][:, t * TQ : (t + 1) * TQ],
                            AF.Square,
                            bias=zero_b[:],
                        )
                        nc.tensor.matmul(
                            ssp[:],
                            r(ones_sb[:]),
                            r(sq[:]),
                            start=(m == 0),
                            stop=(m == nch - 1),
                        )
                    std = wrk.tile([P, TQ], f32, tag="std", name="std")
                    nc.scalar.activation(
                        std[:], ssp[:], AF.Sqrt, bias=eps_b[:], scale=1.0 / fan
                    )
                    rstd = wrk.tile([P, TQ], f32, tag="rstd", name="rstd")
                    with nc.allow_low_precision("rmsnorm rstd"):
                        nc.vector.reciprocal(rstd[:], std[:])
                    for m in range(nch):
                        sl = lat[m][:, t * TQ : (t + 1) * TQ]
                        nc.vector.tensor_tensor(sl, sl, rstd[:], ALU.mult)
                        nc.vector.tensor_scalar_mul(sl, sl, w_sb[:, m : m + 1])

            with tc.tile_pool(name="wqb0p", bufs=1) as wqb0p, \
                    tc.tile_pool(name="qlatp", bufs=1) as qlatp:
                wqb0 = wqb0p.tile([P, QCH, P], f32r, tag="wqb0")
                qlat = [
                    qlatp.tile([P, T], f32r, tag=f"qlat{i}", name=f"qlat{i}")
                    for i in range(QCH)
                ]
                with tc.tile_pool(name="kvlatp", bufs=1) as kvlatp:
                    kvlat = [
                        kvlatp.tile([P, T], f32r, tag=f"kvlat{i}", name=f"kvlat{i}")
                        for i in range(KCH + 1)
                    ]
                    with (
                        tc.tile_pool(name="wkbnp", bufs=1) as wkbnp,
                        tc.tile_pool(name="wkbvp", bufs=1) as wkbvp,
                    ):
                        # stage-D weights: pool reserved up front (no
                        # overlap deps); DMAs emitted mid-stage-A so they
                        # neither delay the first x tiles nor stall D
                        wkn = wkbnp.tile(
                            [P, HG, KCH, P], f32r, tag="wkbn", name="wkbn"
                        )
                        wkvts = [
                            wkbvp.tile(
                                [P, KCH, TQ], f32r, tag=f"wkbv{quad}", name="wkbv"
                            )
                            for quad in range(2)
                        ]

                        def emit_dweight_dmas():
                            nc.sync.dma_start(
                                wkn[:], wkbn[:, :, :, :].bitcast(f32r))
                            for quad in range(2):
                                nc.sync.dma_start(
                                    wkvts[quad][:],
                                    wkbv[quad, :, :, :].bitcast(f32r),
                                )
                            nc.sync.dma_start(
                                wqb0[:], wqbn[0, :, :, :].bitcast(f32r))

                        # ---- stage A: latent projections ----
                        # kv blocks first so the kv path unblocks early
                        # kpe2 lives to the end (right side)
                        kpep = rstack.enter_context(
                            tc.tile_pool(name="kpep", bufs=1, side="right"))
                        kpe2 = kpep.tile([P, T], f32r, tag="kpe2")

                        kvblocks = [
                            (wkva, 0, 256, kvlat, 0),
                            (wkva, 1, 256, kvlat, 2),
                            (wkvr, None, 64, kvlat, 4),
                        ]
                        qblocks = [
                            (wqa, mb, 256, qlat, 2 * mb)
                            for mb in range(QCH // 2)
                        ]

                        wrkA_cm = tc.tile_pool(name="wrkA", bufs=3)
                        wrkA = wrkA_cm.__enter__()
                        with (
                            tc.tile_pool(name="xk", bufs=10) as xkp,
                            tc.tile_pool(name="wA", bufs=3) as wAp,
                        ):
                            for quart in range(4):
                                xk = [None] * KQ
                                mblocks = kvblocks + qblocks
                                for mbi, (wdram, blki, width, dest, dch) in \
                                        enumerate(mblocks):
                                    nm = (width + P - 1) // P
                                    wt = wAp.tile(
                                        [P, KQ, 256], f32r, tag="wA", name="wA"
                                    )
                                    wsrc = (
                                        wdram[quart, :, :, :]
                                        if blki is None
                                        else wdram[blki, quart, :, :, :]
                                    )
                                    first = quart == 0 and mbi == 0
                                    if mbi == 0:
                                        # interleave x and weight loads
                                        for kk in range(KQ):
                                            k = quart * KQ + kk
                                            xt_ = xkp.tile(
                                                [P, T], f32r, tag="xk", name="xk"
                                            )
                                            nc.sync.dma_start(
                                                xt_[:],
                                                xT[k * P : (k + 1) * P, :].bitcast(
                                                    f32r
                                                ),
                                            )
                                            xk[kk] = xt_
                                            if first:
                                                # fine-grained first weight
                                                # loads so the kernel starts
                                                # computing immediately
                                                nc.sync.dma_start(
                                                    wt[:, kk, :width],
                                                    wsrc[:, kk, :].bitcast(f32r),
                                                )
                                                if kk == 0:
                                                    emit_const_dmas()
                                    if not first:
                                        nc.sync.dma_start(
                                            wt[:, :, :width], wsrc.bitcast(f32r)
                                        )
                                    pst = [
                                        [
                                            (psum if (mbi + m_) % 2 == 0 else pssc)
                                            .tile(
                                                [P, TQ],
                                                f32,
                                                tag=(
                                                    "mm"
                                                    if (mbi + m_) % 2 == 0
                                                    else "psc"
                                                ),
                                                name="psA",
                                            )
                                            for _ in range(NT)
                                        ]
                                        for m_ in range(nm)
                                    ]
                                    for m in range(nm):
                                        w = min(width - m * P, P)
                                        for kk in range(KQ):
                                            for t in range(NT):
                                                nc.tensor.matmul(
                                                    pst[m][t][:w, :],
                                                    r(wt[:, kk, m * P : m * P + w]),
                                                    r(
                                                        xk[kk][
                                                            :,
                                                            t * TQ : (t + 1) * TQ,
                                                        ]
                                                    ),
                                                    start=(kk == 0),
                                                    stop=(kk == KQ - 1),
                                                )
                                        for t in range(NT):
                                            dst = dest[dch + m][
                                                :w, t * TQ : (t + 1) * TQ
                                            ]
                                            if quart == 0:
                                                nc.scalar.copy(dst, pst[m][t][:w, :])
                                            else:
                                                nc.vector.tensor_tensor(
                                                    dst,
                                                    pst[m][t][:w, :],
                                                    dst,
                                                    ALU.add,
                                                )
                                    if quart == 1 and mbi == 0:
                                        emit_dweight_dmas()
                                    if quart == 3 and mbi == len(kvblocks) - 1:
                                        # kv latents final: norm + k rope now
                                        # so the kv path completes during the
                                        # remaining q blocks
                                        STAGE_MARKS.append(("A2kv", nc.next_id()))
                                        rmsnorm(kvlat, KCH, kvw_sb, KV_RANK,
                                                pspd, "pd", wrkA)
                                        ksw = xkp.tile(
                                            [P, T], f32r, tag="xk", name="ksw"
                                        )
                                        kswf = ksw[:].bitcast(f32)
                                        nc.sync.dma_start(
                                            kpe2[0:64, :], kvlat[KCH][0:64, :])
                                        nc.sync.dma_start(
                                            kpe2[64:128, :], kvlat[KCH][0:64, :])
                                        nc.sync.dma_start(
                                            kswf[0:32, :],
                                            kvlat[KCH][32:64, :].bitcast(f32))
                                        nc.sync.dma_start(
                                            kswf[32:64, :],
                                            kvlat[KCH][0:32, :].bitcast(f32))
                                        nc.sync.dma_start(
                                            kswf[64:96, :],
                                            kvlat[KCH][32:64, :].bitcast(f32))
                                        nc.sync.dma_start(
                                            kswf[96:128, :],
                                            kvlat[KCH][0:32, :].bitcast(f32))
                                        nc.vector.tensor_tensor(
                                            kswf, kswf, sin_sb[:], ALU.mult)
                                        nc.vector.tensor_tensor(
                                            kpe2[:], kpe2[:], cos_sb[:], ALU.mult)
                                        nc.vector.tensor_tensor(
                                            kpe2[:], kpe2[:], kswf, ALU.add)


                        kTnp = rstack.enter_context(
                            tc.tile_pool(name="kTnp", bufs=1, side="right"))
                        kTn = kTnp.tile([P, HG, T], f32r, tag="kTn")
                        vqp = rstack.enter_context(
                            tc.tile_pool(name="vqp", bufs=2, side="right"))
                        vq = [
                            vqp.tile([P, TC, 4 * D_V], f32r, tag="vq", name="vq")
                            for _ in range(2)
                        ]

                        STAGE_MARKS.append(("D", nc.next_id()))
                        # ---- stage D: kT_nope per head, v per quad ----
                        for h in range(HG):
                            pp, ptag = (psum, "mm") if h % 2 == 0 else (pssc, "psc")
                            pst = [
                                pp.tile([P, TQ], f32, tag=ptag, name="psD")
                                for _ in range(NT)
                            ]
                            for k in range(KCH):
                                for t in range(NT):
                                    nc.tensor.matmul(
                                        pst[t][:],
                                        r(wkn[:, h, k, :]),
                                        r(kvlat[k][:, t * TQ : (t + 1) * TQ]),
                                        start=(k == 0),
                                        stop=(k == KCH - 1),
                                    )
                            for t in range(NT):
                                nc.vector.tensor_copy(
                                    kTn[:, h, t * TQ : (t + 1) * TQ], pst[t][:]
                                )
                        for quad in range(2):
                            for tkc in range(TC):
                                pp, ptag = (
                                    (psum, "mm") if tkc % 2 == 0 else (pssc, "psc")
                                )
                                ps_ = pp.tile([P, TQ], f32, tag=ptag, name="psV")
                                for k in range(KCH):
                                    nc.tensor.matmul(
                                        ps_[:],
                                        r(kvlat[k][:, tkc * P : (tkc + 1) * P]),
                                        r(wkvts[quad][:, k, :]),
                                        start=(k == 0),
                                        stop=(k == KCH - 1),
                                    )
                                nc.vector.tensor_copy(vq[quad][:, tkc, :], ps_[:])

                        STAGE_MARKS.append(("A2q", nc.next_id()))
                        rmsnorm(qlat, QCH, qw_sb, Q_RANK, pssc, "psc", wrkA)
                        wrkA_cm.__exit__(None, None, None)

                # kvlat + stage-D weight pools closed here
                actq = rstack.enter_context(
                    tc.tile_pool(name="actq", bufs=1, side="right"))
                qTn = actq.tile([P, HG, T], f32r, tag="qTn")
                qTr = actq.tile([P, HG // 2, T], f32r, tag="qTr")

                STAGE_MARKS.append(("B", nc.next_id()))
                # ---- stage B: qT per head (one DMA per head/pair) ----
                with (
                    tc.tile_pool(name="wqbp", bufs=2) as wqbp,
                    tc.tile_pool(name="qswp", bufs=1) as qswp,
                ):
                    for h in range(HG):
                        if h == 0:
                            wt = wqb0
                        else:
                            wt = wqbp.tile(
                                [P, QCH, P], f32r, tag="wqb", name="wqbn")
                            nc.sync.dma_start(
                                wt[:], wqbn[h, :, :, :].bitcast(f32r))
                        pp, ptag = (psum, "mm") if h % 2 == 0 else (pssc, "psc")
                        pst = [
                            pp.tile([P, TQ], f32, tag=ptag, name="psB")
                            for _ in range(NT)
                        ]
                        for k in range(QCH):
                            for t in range(NT):
                                nc.tensor.matmul(
                                    pst[t][:],
                                    r(wt[:, k, :]),
                                    r(qlat[k][:, t * TQ : (t + 1) * TQ]),
                                    start=(k == 0),
                                    stop=(k == QCH - 1),
                                )
                        for t in range(NT):
                            nc.vector.tensor_copy(
                                qTn[:, h, t * TQ : (t + 1) * TQ], pst[t][:]
                            )
                    for p_ in range(HG // 2):
                        wt = wqbp.tile([P, QCH, P], f32r, tag="wqb", name="wqbr")
                        nc.sync.dma_start(wt[:], wqbr[p_, :, :, :].bitcast(f32r))
                        pp, ptag = (psum, "mm") if p_ % 2 == 0 else (pssc, "psc")
                        pst = [
                            pp.tile([P, TQ], f32, tag=ptag, name="psB2")
                            for _ in range(NT)
                        ]
                        for k in range(QCH):
                            for t in range(NT):
                                nc.tensor.matmul(
                                    pst[t][:],
                                    r(wt[:, k, :]),
                                    r(qlat[k][:, t * TQ : (t + 1) * TQ]),
                                    start=(k == 0),
                                    stop=(k == QCH - 1),
                                )
                        for t in range(NT):
                            nc.vector.tensor_copy(
                                qTr[:, p_, t * TQ : (t + 1) * TQ], pst[t][:]
                            )
                        # rope this pair immediately (overlaps next pair)
                        qsw = qswp.tile([P, T], f32, tag="qsw", name="qsw")
                        qp = qTr[:, p_, :]
                        nc.sync.dma_start(qsw[0:32, :], qp[32:64, :].bitcast(f32))
                        nc.sync.dma_start(qsw[32:64, :], qp[0:32, :].bitcast(f32))
                        nc.sync.dma_start(qsw[64:96, :], qp[96:128, :].bitcast(f32))
                        nc.sync.dma_start(qsw[96:128, :], qp[64:96, :].bitcast(f32))
                        nc.vector.tensor_tensor(qsw[:], qsw[:], sin_sb[:], ALU.mult)
                        nc.vector.tensor_tensor(qp, qp, cos_sb[:], ALU.mult)
                        nc.vector.tensor_tensor(qp, qp, qsw[:], ALU.add)

            STAGE_MARKS.append(("EF", nc.next_id()))
            # kvlat + qlat pools closed here
            # ---- stages E+F per token tile (t=1 first: its leading tk
            # chunks need no causal mask, hiding the mask DMA) ----
            with (
                tc.tile_pool(name="cmp", bufs=1) as cmp_,
                tc.tile_pool(name="attp", bufs=1) as attp,
                tc.tile_pool(name="wrkE", bufs=3) as wrkE,
                tc.tile_pool(name="recp", bufs=2) as recp,
                tc.tile_pool(name="wop", bufs=3) as wop,
                tc.tile_pool(name="outs", bufs=2) as outp,
            ):
                # sliding causal mask: cm[dk, u] = 0 iff dk <= u - 384;
                # chunk variant rv uses columns [(3-rv)*128, (3-rv)*128+512)
                cm_sb = cmp_.tile([P, 7 * P], f32, tag="cm")
                nc.sync.dma_start(cm_sb[:], cmask[:, :])
                for t in (1, 0):
                    attnT = attp.tile([P, HG, TQ], f32r, tag="attnT", name="attnT")
                    nchunks = 4 * (t + 1)
                    for h in range(HG):
                        hb = 64 * (h % 2)
                        pd = pspd.tile([P, TQ], f32, tag="pd", name="pd")
                        pav = psum.tile([P, TQ], f32, tag="mm", name="pav")
                        for tkc in range(nchunks):
                            ps_ = pssc.tile([P, TQ], f32, tag="psc", name="psc")
                            nc.tensor.matmul(
                                ps_[:],
                                r(kTn[:, h, tkc * P : (tkc + 1) * P]),
                                r(qTn[:, h, t * TQ : (t + 1) * TQ]),
                                start=True,
                                stop=False,
                            )
                            nc.tensor.matmul(
                                ps_[:],
                                r(kpe2[hb : hb + 64, tkc * P : (tkc + 1) * P]),
                                r(
                                    qTr[
                                        hb : hb + 64,
                                        h // 2,
                                        t * TQ : (t + 1) * TQ,
                                    ]
                                ),
                                start=False,
                                stop=True,
                            )
                            if tkc >= 4 * t:
                                rv = tkc - 4 * t
                                nc.vector.tensor_tensor(
                                    ps_[:],
                                    ps_[:],
                                    cm_sb[:, (3 - rv) * P : (3 - rv) * P + TQ],
                                    ALU.add,
                                )
                            ex = wrkE.tile([P, TQ], f32r, tag="exp", name="ex")
                            nc.scalar.activation(
                                ex[:],
                                ps_[:],
                                AF.Exp,
                                bias=kb_sb[:, tkc : tkc + 1],
                                scale=SCALE,
                            )
                            nc.tensor.matmul(
                                pd[:],
                                r(ones_sb[:]),
                                r(ex[:]),
                                start=(tkc == 0),
                                stop=(tkc == nchunks - 1),
                            )
                            nc.tensor.matmul(
                                pav[:],
                                r(
                                    vq[h // 4][
                                        :, tkc, (h % 4) * P : (h % 4 + 1) * P
                                    ]
                                ),
                                r(ex[:]),
                                start=(tkc == 0),
                                stop=(tkc == nchunks - 1),
                            )
                        rec = recp.tile([P, TQ], f32, tag="rec", name="rec")
                        with nc.allow_low_precision("softmax denom"):
                            nc.vector.reciprocal(rec[:], pd[:])
                        nc.vector.tensor_tensor(
                            attnT[:, h, :], pav[:], rec[:], ALU.mult
                        )
                    # output projection for this token tile
                    for nt in range(NHID):
                        wts = []
                        for half in range(2):
                            wt = wop.tile([P, 4, TQ], f32r, tag="wo", name="wo")
                            nc.sync.dma_start(
                                wt[:], wo[nt, half, :, :, :].bitcast(f32r)
                            )
                            wts.append(wt)
                        for tqc in range(TQ // P):
                            po = psum.tile([P, TQ], f32, tag="mm", name="po")
                            for h in range(HG):
                                nc.tensor.matmul(
                                    po[:],
                                    r(attnT[:, h, tqc * P : (tqc + 1) * P]),
                                    r(wts[h // 4][:, h % 4, :]),
                                    start=(h == 0),
                                    stop=(h == HG - 1),
                                )
                            ot = outp.tile([P, TQ], f32, tag="osb", name="ot")
                            nc.vector.tensor_copy(ot[:], po[:])
                            nc.sync.dma_start(out[t, nt, tqc, :, :], ot[:])

    nc.finalize()
    return nc


def _get_program():
    if "nc" not in _CACHED:
        _CACHED["nc"] = _build_program()
    return _CACHED["nc"]


def _host_prep(x, wq_a, q_norm_w, wq_b, wkv_a, kv_norm_w, wkv_b, wo,
               attention_mask, positions):
    """Build the 8 per-core input maps.

    All weight tensors are host-packed partition-major so every device
    DMA is one large contiguous transfer.
    """
    f = np.float32
    x = np.asarray(x, f)
    wq_a = np.asarray(wq_a, f)
    wkv_a = np.asarray(wkv_a, f)
    wq_b3 = np.asarray(wq_b, f).reshape(Q_RANK, H, D_QK)
    wkv_b3 = np.asarray(wkv_b, f).reshape(KV_RANK, H, D_NOPE + D_V)
    wo2 = np.asarray(wo, f)
    q_norm_w = np.asarray(q_norm_w, f)
    kv_norm_w = np.asarray(kv_norm_w, f)
    attention_mask = np.asarray(attention_mask)
    positions = np.asarray(positions)

    qw = np.ascontiguousarray(q_norm_w.reshape(QCH, P).T)
    kvw = np.ascontiguousarray(kv_norm_w.reshape(KCH, P).T)

    # stage-A weights: [block, quart, p, kk, width] (replicated on all cores)
    wqa_blk = np.ascontiguousarray(
        wq_a.reshape(4, KQ, P, 6, 256).transpose(3, 0, 2, 1, 4))
    wkva_blk = np.ascontiguousarray(
        wkv_a[:, :KV_RANK].reshape(4, KQ, P, 2, 256).transpose(3, 0, 2, 1, 4))
    wkvr_blk = np.ascontiguousarray(
        wkv_a[:, KV_RANK:].reshape(4, KQ, P, 64).transpose(0, 2, 1, 3))

    inv_freq = 1.0 / (THETA ** (np.arange(0, D_ROPE, 2, dtype=np.float64) / D_ROPE))

    # sliding causal mask: cm[dk, u] = 0 iff dk <= u - 384
    dk = np.arange(P)[:, None]
    u = np.arange(7 * P)[None, :]
    cmask = np.where(dk <= u - 3 * P, 0.0, NMASK).astype(f)
    onesd = np.ones((P, P), f)

    per_batch = {}
    for b in range(B):
        xTb = np.ascontiguousarray(x[b].T)
        ang = positions[b].astype(np.float64)[:, None] * inv_freq[None, :]
        cosT = np.cos(ang).astype(f).T  # [32, T]
        sinT = np.sin(ang).astype(f).T
        cos4 = np.ascontiguousarray(np.tile(cosT, (4, 1)))
        sin4 = np.ascontiguousarray(
            np.concatenate([-sinT, sinT, -sinT, sinT], axis=0))
        kb = np.where(attention_mask[b] != 0, 0.0, NMASK).astype(f)
        kbias = np.ascontiguousarray(kb.reshape(TC, P).T)
        per_batch[b] = (xTb, cos4, sin4, kbias)

    in_maps = []
    for c in range(8):
        b, g = c // 4, c % 4
        hs = slice(g * HG, (g + 1) * HG)
        xTb, cos4, sin4, kbias = per_batch[b]
        # [rank, head, dim] -> [h, p, k, c] / [pair, p, k, c]
        wqbn_ = np.ascontiguousarray(
            wq_b3[:, hs, :D_NOPE]
            .reshape(QCH, P, HG, P).transpose(2, 1, 0, 3))
        # rope cols packed in head pairs: [h_even 64 | h_odd 64] per 128-col
        wqbr_ = np.ascontiguousarray(
            wq_b3[:, hs, D_NOPE:]
            .reshape(QCH, P, HG // 2, P).transpose(2, 1, 0, 3))
        wkbn_ = np.ascontiguousarray(
            wkv_b3[:, hs, :D_NOPE]
            .reshape(KCH, P, HG, P).transpose(1, 2, 0, 3))
        wkbv_ = np.ascontiguousarray(
            wkv_b3[:, hs, D_NOPE:]
            .reshape(KCH, P, 2, TQ).transpose(2, 1, 0, 3))
        # wo rows h*128+dv, cols nt*512+c -> [nt, half, dv(p), h%4, c]
        wosh = np.ascontiguousarray(
            wo2[g * HG * D_V : (g + 1) * HG * D_V, :]
            .reshape(2, 4, P, NHID, TQ).transpose(3, 0, 2, 1, 4))
        in_maps.append({
            "xT": xTb, "wqa": wqa_blk, "wkva": wkva_blk, "wkvr": wkvr_blk,
            "wqbn": wqbn_, "wqbr": wqbr_, "wkbn": wkbn_, "wkbv": wkbv_,
            "wo": wosh, "qw": qw, "kvw": kvw,
            "cos4": cos4, "sin4": sin4,
            "cmask": cmask, "kbias": kbias, "onesd": onesd,
        })
    return in_maps


def kernel(**inputs):
    from concourse.bass_utils import run_bass_kernel_spmd

    nc = _get_program()
    in_maps = _host_prep(**inputs)
    res = run_bass_kernel_spmd(nc, in_maps, core_ids=list(range(8)))
    _CACHED["last_result"] = res
    out = np.zeros((B, T, HID), np.float32)
    for c in range(8):
        blk = res.results[c]["out"]  # [NT, NHID, 4, P, TQ]
        # row = t*512 + q*128 + p, col = nt*512 + c
        out[c // 4] += blk.transpose(0, 2, 3, 1, 4).reshape(T, HID)
    return out


# revision 26
# speedup vs baseline: 1.3722x; 1.0172x over previous
"""DeepseekV3 MLA forward on 8 TRN2 NeuronCores.

Sharding: data-parallel over batch (B=2 -> 2 groups of 4 cores), tensor-
parallel over heads within each batch group (32 heads -> 4 groups of 8).
Each core computes its batch element's full latent projections (wq_a /
wkv_a replicated), its 8 heads' q/k/v expansions + attention, and a
partial output projection (wo row-shard); the host sums the 4 partial
outputs per batch element.

Dataflow on device keeps activations transposed ([feature, token]) so
every matmul contracts over the partition dim with no on-device
transposes anywhere:
  latentT = wq_a.T @ xT        (lhsT=wq_a cols, rhs=xT)        [rank, T]
  qT_h    = wq_b_h.T @ latentT                                  [d, T]
  scoresT = kT_h-chunks @ qT_h                                  [tk, tq]
  softmax over tk (=partitions): exp on ACT, denominator via a
  ones[128,128] matmul (result replicated across partitions), then
  attn_outT = v_chunks.T @ expT                                 [dv, tq]
  out      = attnT-chunks.T @ wo_h  (natural layout)            [tq, hid]
RoPE in transposed layout: rot(x) = x*cos + swap32(x)*(+-sin), where
swap32 exchanges the two 32-row halves of each 64-row rope block (done
with SBUF->SBUF block DMAs) and the +-sin sign pattern is host-built.
All matmuls run as float32r (FP22-truncated fp32, single pass).

Perf notes: per-DMA issue costs ~1us on the sync sequencer, so weights
are host-packed into large partition-major blocks and DMA'd in few big
transfers (SBUF slices feed the matmuls). SBUF pools are statically
reserved; they are scoped per stage, long-lived activation pools on the
"right" allocator side. PSUM: 8 banks split mm:4 / pd:2 / sc:2.
"""

import os
import sys

import numpy as np

sys.path.insert(0, "/opt/trn_rl_repo")

B, T, HID = 2, 1024, 4096
H, D_NOPE, D_ROPE, D_V = 32, 128, 64, 128
D_QK = D_NOPE + D_ROPE
Q_RANK, KV_RANK = 1536, 512
THETA, EPS = 10000.0, 1e-6
SCALE = float(D_QK) ** -0.5
NMASK = -30000.0

HG = H // 4          # heads per core = 8
P = 128
QCH = Q_RANK // P    # 12 latent chunks (q)
KCH = KV_RANK // P   # 4 latent chunks (kv)
HIDK = HID // P      # 32 contraction tiles for stage A
KQ = HIDK // 4       # 8 k-tiles per stage-A quarter
TQ = 512             # token tile (free dim) for most matmuls
NT = T // TQ         # 2 token tiles
TC = T // P          # 8 token chunks of 128
NHID = HID // TQ     # 8 output column tiles

_CACHED = {}
STAGE_MARKS = []


def _build_program():
    import contextlib

    import concourse.bacc as bacc
    import concourse.mybir as mybir
    import concourse.tile as tile

    f32 = mybir.dt.float32
    f32r = mybir.dt.float32r
    AF = mybir.ActivationFunctionType
    ALU = mybir.AluOpType

    nc = bacc.Bacc()

    # ---- DRAM I/O (per-core shapes; SPMD across the 8 cores) ----
    # weights are host-packed partition-major so each DMA is one big
    # contiguous transfer whose SBUF image is sliced per matmul
    xT = nc.dram_tensor("xT", (HID, T), f32, kind="ExternalInput")
    wqa = nc.dram_tensor("wqa", (6, 4, P, KQ, 256), f32, kind="ExternalInput")
    wkva = nc.dram_tensor("wkva", (2, 4, P, KQ, 256), f32, kind="ExternalInput")
    wkvr = nc.dram_tensor("wkvr", (4, P, KQ, 64), f32, kind="ExternalInput")
    wqbn = nc.dram_tensor("wqbn", (HG, P, QCH, P), f32, kind="ExternalInput")
    wqbr = nc.dram_tensor("wqbr", (HG // 2, P, QCH, P), f32, kind="ExternalInput")
    wkbn = nc.dram_tensor("wkbn", (P, HG, KCH, P), f32, kind="ExternalInput")
    wkbv = nc.dram_tensor("wkbv", (2, P, KCH, TQ), f32, kind="ExternalInput")
    wo = nc.dram_tensor("wo", (NHID, 2, P, 4, TQ), f32, kind="ExternalInput")
    qw = nc.dram_tensor("qw", (P, QCH), f32, kind="ExternalInput")
    kvw = nc.dram_tensor("kvw", (P, KCH), f32, kind="ExternalInput")
    cos4 = nc.dram_tensor("cos4", (P, T), f32, kind="ExternalInput")
    sin4 = nc.dram_tensor("sin4", (P, T), f32, kind="ExternalInput")  # +-sin
    cmask = nc.dram_tensor("cmask", (P, 7 * P), f32, kind="ExternalInput")
    kbias = nc.dram_tensor("kbias", (P, TC), f32, kind="ExternalInput")
    onesd = nc.dram_tensor("onesd", (P, P), f32, kind="ExternalInput")
    out = nc.dram_tensor("out", (NT, NHID, 4, P, TQ), f32, kind="ExternalOutput")

    def r(ap):
        return ap.bitcast(f32r)

    with tile.TileContext(nc) as tc, contextlib.ExitStack() as rstack:
        with (
            tc.tile_pool(name="const", bufs=1) as const,
            tc.tile_pool(name="psmm", bufs=3, space="PSUM") as psum,
            tc.tile_pool(name="pspd", bufs=2, space="PSUM") as pspd,
            tc.tile_pool(name="pssc", bufs=3, space="PSUM") as pssc,
        ):
            # ---- constants (persistent; DMAs deferred past the first
            # stage-A tiles so they don't delay the first matmuls) ----
            ones_sb = const.tile([P, P], f32r, tag="ones")
            qw_sb = const.tile([P, QCH], f32, tag="qw")
            kvw_sb = const.tile([P, KCH], f32, tag="kvw")
            cos_sb = const.tile([P, T], f32, tag="cos")
            sin_sb = const.tile([P, T], f32, tag="sin")
            kb_sb = const.tile([P, TC], f32, tag="kb")
            zero_b = const.tile([P, 1], f32, tag="zb")
            nc.vector.memset(zero_b[:], 0.0)
            eps_b = const.tile([P, 1], f32, tag="eb")
            nc.vector.memset(eps_b[:], EPS)

            def emit_const_dmas():
                nc.sync.dma_start(ones_sb[:], onesd[:, :].bitcast(f32r))
                nc.sync.dma_start(qw_sb[:], qw[:, :])
                nc.sync.dma_start(kvw_sb[:], kvw[:, :])
                nc.sync.dma_start(cos_sb[:], cos4[:, :])
                nc.sync.dma_start(sin_sb[:], sin4[:, :])
                nc.sync.dma_start(kb_sb[:], kbias[:, :])

            def rmsnorm(lat, nch, w_sb, fan, sspool, sstag, wrk):
                for t in range(NT):
                    ssp = sspool.tile([P, TQ], f32, tag=sstag, name="ssp")
                    for m in range(nch):
                        sq = wrk.tile([P, TQ], f32r, tag="sq", name="sq")
                        nc.scalar.activation(
                            sq[:],
                            lat[m][:, t * TQ : (t + 1) * TQ],
                            AF.Square,
                            bias=zero_b[:],
                        )
                        nc.tensor.matmul(
                            ssp[:],
                            r(ones_sb[:]),
                            r(sq[:]),
                            start=(m == 0),
                            stop=(m == nch - 1),
                        )
                    std = wrk.tile([P, TQ], f32, tag="std", name="std")
                    nc.scalar.activation(
                        std[:], ssp[:], AF.Sqrt, bias=eps_b[:], scale=1.0 / fan
                    )
                    rstd = wrk.tile([P, TQ], f32, tag="rstd", name="rstd")
                    with nc.allow_low_precision("rmsnorm rstd"):
                        nc.vector.reciprocal(rstd[:], std[:])
                    for m in range(nch):
                        sl = lat[m][:, t * TQ : (t + 1) * TQ]
                        nc.vector.tensor_tensor(sl, sl, rstd[:], ALU.mult)
                        nc.vector.tensor_scalar_mul(sl, sl, w_sb[:, m : m + 1])

            with tc.tile_pool(name="wqb0p", bufs=1) as wqb0p, \
                    tc.tile_pool(name="qlatp", bufs=1) as qlatp:
                wqb0 = wqb0p.tile([P, QCH, P], f32r, tag="wqb0")
                qlat = [
                    qlatp.tile([P, T], f32r, tag=f"qlat{i}", name=f"qlat{i}")
                    for i in range(QCH)
                ]
                with tc.tile_pool(name="kvlatp", bufs=1) as kvlatp:
                    kvlat = [
                        kvlatp.tile([P, T], f32r, tag=f"kvlat{i}", name=f"kvlat{i}")
                        for i in range(KCH + 1)
                    ]
                    with (
                        tc.tile_pool(name="wkbnp", bufs=1) as wkbnp,
                        tc.tile_pool(name="wkbvp", bufs=1) as wkbvp,
                    ):
                        # stage-D weights: pool reserved up front (no
                        # overlap deps); DMAs emitted mid-stage-A so they
                        # neither delay the first x tiles nor stall D
                        wkn = wkbnp.tile(
                            [P, HG, KCH, P], f32r, tag="wkbn", name="wkbn"
                        )
                        wkvts = [
                            wkbvp.tile(
                                [P, KCH, TQ], f32r, tag=f"wkbv{quad}", name="wkbv"
                            )
                            for quad in range(2)
                        ]

                        def emit_dweight_dmas():
                            nc.sync.dma_start(
                                wkn[:], wkbn[:, :, :, :].bitcast(f32r))
                            for quad in range(2):
                                nc.sync.dma_start(
                                    wkvts[quad][:],
                                    wkbv[quad, :, :, :].bitcast(f32r),
                                )
                            nc.sync.dma_start(
                                wqb0[:], wqbn[0, :, :, :].bitcast(f32r))

                        # ---- stage A: latent projections ----
                        # kv blocks first so the kv path unblocks early
                        # kpe2 lives to the end (right side)
                        kpep = rstack.enter_context(
                            tc.tile_pool(name="kpep", bufs=1, side="right"))
                        kpe2 = kpep.tile([P, T], f32r, tag="kpe2")

                        kvblocks = [
                            (wkva, 0, 256, kvlat, 0),
                            (wkva, 1, 256, kvlat, 2),
                            (wkvr, None, 64, kvlat, 4),
                        ]
                        qblocks = [
                            (wqa, mb, 256, qlat, 2 * mb)
                            for mb in range(QCH // 2)
                        ]

                        wrkA_cm = tc.tile_pool(name="wrkA", bufs=3)
                        wrkA = wrkA_cm.__enter__()
                        with (
                            tc.tile_pool(name="xk", bufs=10) as xkp,
                            tc.tile_pool(name="wA", bufs=3) as wAp,
                        ):
                            for quart in range(4):
                                xk = [None] * KQ
                                mblocks = kvblocks + qblocks
                                for mbi, (wdram, blki, width, dest, dch) in \
                                        enumerate(mblocks):
                                    nm = (width + P - 1) // P
                                    wt = wAp.tile(
                                        [P, KQ, 256], f32r, tag="wA", name="wA"
                                    )
                                    wsrc = (
                                        wdram[quart, :, :, :]
                                        if blki is None
                                        else wdram[blki, quart, :, :, :]
                                    )
                                    first = quart == 0 and mbi <= 1
                                    if mbi == 0:
                                        # interleave x and weight loads
                                        for kk in range(KQ):
                                            k = quart * KQ + kk
                                            xt_ = xkp.tile(
                                                [P, T], f32r, tag="xk", name="xk"
                                            )
                                            nc.sync.dma_start(
                                                xt_[:],
                                                xT[k * P : (k + 1) * P, :].bitcast(
                                                    f32r
                                                ),
                                            )
                                            xk[kk] = xt_
                                            if first:
                                                # fine-grained first weight
                                                # loads so the kernel starts
                                                # computing immediately
                                                nc.sync.dma_start(
                                                    wt[:, kk, :width],
                                                    wsrc[:, kk, :].bitcast(f32r),
                                                )
                                                if kk == 0:
                                                    emit_const_dmas()
                                    if first and mbi == 1:
                                        for kk in range(KQ):
                                            nc.sync.dma_start(
                                                wt[:, kk, :width],
                                                wsrc[:, kk, :].bitcast(f32r),
                                            )
                                    if not first:
                                        nc.sync.dma_start(
                                            wt[:, :, :width], wsrc.bitcast(f32r)
                                        )
                                    pst = [
                                        [
                                            (psum if (mbi + m_) % 2 == 0 else pssc)
                                            .tile(
                                                [P, TQ],
                                                f32,
                                                tag=(
                                                    "mm"
                                                    if (mbi + m_) % 2 == 0
                                                    else "psc"
                                                ),
                                                name="psA",
                                            )
                                            for _ in range(NT)
                                        ]
                                        for m_ in range(nm)
                                    ]
                                    for m in range(nm):
                                        w = min(width - m * P, P)
                                        for kk in range(KQ):
                                            for t in range(NT):
                                                nc.tensor.matmul(
                                                    pst[m][t][:w, :],
                                                    r(wt[:, kk, m * P : m * P + w]),
                                                    r(
                                                        xk[kk][
                                                            :,
                                                            t * TQ : (t + 1) * TQ,
                                                        ]
                                                    ),
                                                    start=(kk == 0),
                                                    stop=(kk == KQ - 1),
                                                )
                                        for t in range(NT):
                                            dst = dest[dch + m][
                                                :w, t * TQ : (t + 1) * TQ
                                            ]
                                            if quart == 0:
                                                nc.scalar.copy(dst, pst[m][t][:w, :])
                                            else:
                                                nc.vector.tensor_tensor(
                                                    dst,
                                                    pst[m][t][:w, :],
                                                    dst,
                                                    ALU.add,
                                                )
                                    if quart == 1 and mbi == 0:
                                        emit_dweight_dmas()
                                    if quart == 3 and mbi == len(kvblocks) - 1:
                                        # kv latents final: norm + k rope now
                                        # so the kv path completes during the
                                        # remaining q blocks
                                        STAGE_MARKS.append(("A2kv", nc.next_id()))
                                        rmsnorm(kvlat, KCH, kvw_sb, KV_RANK,
                                                pspd, "pd", wrkA)
                                        ksw = xkp.tile(
                                            [P, T], f32r, tag="xk", name="ksw"
                                        )
                                        kswf = ksw[:].bitcast(f32)
                                        nc.sync.dma_start(
                                            kpe2[0:64, :], kvlat[KCH][0:64, :])
                                        nc.sync.dma_start(
                                            kpe2[64:128, :], kvlat[KCH][0:64, :])
                                        nc.sync.dma_start(
                                            kswf[0:32, :],
                                            kvlat[KCH][32:64, :].bitcast(f32))
                                        nc.sync.dma_start(
                                            kswf[32:64, :],
                                            kvlat[KCH][0:32, :].bitcast(f32))
                                        nc.sync.dma_start(
                                            kswf[64:96, :],
                                            kvlat[KCH][32:64, :].bitcast(f32))
                                        nc.sync.dma_start(
                                            kswf[96:128, :],
                                            kvlat[KCH][0:32, :].bitcast(f32))
                                        nc.vector.tensor_tensor(
                                            kswf, kswf, sin_sb[:], ALU.mult)
                                        nc.vector.tensor_tensor(
                                            kpe2[:], kpe2[:], cos_sb[:], ALU.mult)
                                        nc.vector.tensor_tensor(
                                            kpe2[:], kpe2[:], kswf, ALU.add)


                        kTnp = rstack.enter_context(
                            tc.tile_pool(name="kTnp", bufs=1, side="right"))
                        kTn = kTnp.tile([P, HG, T], f32r, tag="kTn")
                        vqp = rstack.enter_context(
                            tc.tile_pool(name="vqp", bufs=2, side="right"))
                        vq = [
                            vqp.tile([P, TC, 4 * D_V], f32r, tag="vq", name="vq")
                            for _ in range(2)
                        ]

                        STAGE_MARKS.append(("D", nc.next_id()))
                        # ---- stage D: kT_nope per head, v per quad ----
                        for h in range(HG):
                            pp, ptag = (psum, "mm") if h % 2 == 0 else (pssc, "psc")
                            pst = [
                                pp.tile([P, TQ], f32, tag=ptag, name="psD")
                                for _ in range(NT)
                            ]
                            for k in range(KCH):
                                for t in range(NT):
                                    nc.tensor.matmul(
                                        pst[t][:],
                                        r(wkn[:, h, k, :]),
                                        r(kvlat[k][:, t * TQ : (t + 1) * TQ]),
                                        start=(k == 0),
                                        stop=(k == KCH - 1),
                                    )
                            for t in range(NT):
                                nc.vector.tensor_copy(
                                    kTn[:, h, t * TQ : (t + 1) * TQ], pst[t][:]
                                )
                        for quad in range(2):
                            for tkc in range(TC):
                                pp, ptag = (
                                    (psum, "mm") if tkc % 2 == 0 else (pssc, "psc")
                                )
                                ps_ = pp.tile([P, TQ], f32, tag=ptag, name="psV")
                                for k in range(KCH):
                                    nc.tensor.matmul(
                                        ps_[:],
                                        r(kvlat[k][:, tkc * P : (tkc + 1) * P]),
                                        r(wkvts[quad][:, k, :]),
                                        start=(k == 0),
                                        stop=(k == KCH - 1),
                                    )
                                nc.vector.tensor_copy(vq[quad][:, tkc, :], ps_[:])

                        STAGE_MARKS.append(("A2q", nc.next_id()))
                        rmsnorm(qlat, QCH, qw_sb, Q_RANK, pssc, "psc", wrkA)
                        wrkA_cm.__exit__(None, None, None)

                # kvlat + stage-D weight pools closed here
                actq = rstack.enter_context(
                    tc.tile_pool(name="actq", bufs=1, side="right"))
                qTn = actq.tile([P, HG, T], f32r, tag="qTn")
                qTr = actq.tile([P, HG // 2, T], f32r, tag="qTr")

                STAGE_MARKS.append(("B", nc.next_id()))
                # ---- stage B: qT per head (one DMA per head/pair) ----
                with (
                    tc.tile_pool(name="wqbp", bufs=3) as wqbp,
                    tc.tile_pool(name="qswp", bufs=1) as qswp,
                ):
                    for h in range(HG):
                        if h == 0:
                            wt = wqb0
                        else:
                            wt = wqbp.tile(
                                [P, QCH, P], f32r, tag="wqb", name="wqbn")
                            nc.sync.dma_start(
                                wt[:], wqbn[h, :, :, :].bitcast(f32r))
                        pp, ptag = (psum, "mm") if h % 2 == 0 else (pssc, "psc")
                        pst = [
                            pp.tile([P, TQ], f32, tag=ptag, name="psB")
                            for _ in range(NT)
                        ]
                        for k in range(QCH):
                            for t in range(NT):
                                nc.tensor.matmul(
                                    pst[t][:],
                                    r(wt[:, k, :]),
                                    r(qlat[k][:, t * TQ : (t + 1) * TQ]),
                                    start=(k == 0),
                                    stop=(k == QCH - 1),
                                )
                        for t in range(NT):
                            nc.vector.tensor_copy(
                                qTn[:, h, t * TQ : (t + 1) * TQ], pst[t][:]
                            )
                    for p_ in range(HG // 2):
                        wt = wqbp.tile([P, QCH, P], f32r, tag="wqb", name="wqbr")
                        nc.sync.dma_start(wt[:], wqbr[p_, :, :, :].bitcast(f32r))
                        pp, ptag = (psum, "mm") if p_ % 2 == 0 else (pssc, "psc")
                        pst = [
                            pp.tile([P, TQ], f32, tag=ptag, name="psB2")
                            for _ in range(NT)
                        ]
                        for k in range(QCH):
                            for t in range(NT):
                                nc.tensor.matmul(
                                    pst[t][:],
                                    r(wt[:, k, :]),
                                    r(qlat[k][:, t * TQ : (t + 1) * TQ]),
                                    start=(k == 0),
                                    stop=(k == QCH - 1),
                                )
                        for t in range(NT):
                            nc.vector.tensor_copy(
                                qTr[:, p_, t * TQ : (t + 1) * TQ], pst[t][:]
                            )
                        # rope this pair immediately (overlaps next pair)
                        qsw = qswp.tile([P, T], f32, tag="qsw", name="qsw")
                        qp = qTr[:, p_, :]
                        nc.sync.dma_start(qsw[0:32, :], qp[32:64, :].bitcast(f32))
                        nc.sync.dma_start(qsw[32:64, :], qp[0:32, :].bitcast(f32))
                        nc.sync.dma_start(qsw[64:96, :], qp[96:128, :].bitcast(f32))
                        nc.sync.dma_start(qsw[96:128, :], qp[64:96, :].bitcast(f32))
                        nc.vector.tensor_tensor(qsw[:], qsw[:], sin_sb[:], ALU.mult)
                        nc.vector.tensor_tensor(qp, qp, cos_sb[:], ALU.mult)
                        nc.vector.tensor_tensor(qp, qp, qsw[:], ALU.add)

            STAGE_MARKS.append(("EF", nc.next_id()))
            # kvlat + qlat pools closed here
            # ---- stages E+F per token tile (t=1 first: its leading tk
            # chunks need no causal mask, hiding the mask DMA) ----
            with (
                tc.tile_pool(name="cmp", bufs=1) as cmp_,
                tc.tile_pool(name="attp", bufs=1) as attp,
                tc.tile_pool(name="wrkE", bufs=3) as wrkE,
                tc.tile_pool(name="recp", bufs=2) as recp,
                tc.tile_pool(name="wop", bufs=3) as wop,
                tc.tile_pool(name="outs", bufs=2) as outp,
            ):
                # sliding causal mask: cm[dk, u] = 0 iff dk <= u - 384;
                # chunk variant rv uses columns [(3-rv)*128, (3-rv)*128+512)
                cm_sb = cmp_.tile([P, 7 * P], f32, tag="cm")
                nc.sync.dma_start(cm_sb[:], cmask[:, :])
                for t in (1, 0):
                    attnT = attp.tile([P, HG, TQ], f32r, tag="attnT", name="attnT")
                    nchunks = 4 * (t + 1)
                    for h in range(HG):
                        hb = 64 * (h % 2)
                        pd = pspd.tile([P, TQ], f32, tag="pd", name="pd")
                        pav = psum.tile([P, TQ], f32, tag="mm", name="pav")
                        for tkc in range(nchunks):
                            ps_ = pssc.tile([P, TQ], f32, tag="psc", name="psc")
                            nc.tensor.matmul(
                                ps_[:],
                                r(kTn[:, h, tkc * P : (tkc + 1) * P]),
                                r(qTn[:, h, t * TQ : (t + 1) * TQ]),
                                start=True,
                                stop=False,
                            )
                            nc.tensor.matmul(
                                ps_[:],
                                r(kpe2[hb : hb + 64, tkc * P : (tkc + 1) * P]),
                                r(
                                    qTr[
                                        hb : hb + 64,
                                        h // 2,
                                        t * TQ : (t + 1) * TQ,
                                    ]
                                ),
                                start=False,
                                stop=True,
                            )
                            if tkc >= 4 * t:
                                rv = tkc - 4 * t
                                nc.vector.tensor_tensor(
                                    ps_[:],
                                    ps_[:],
                                    cm_sb[:, (3 - rv) * P : (3 - rv) * P + TQ],
                                    ALU.add,
                                )
                            ex = wrkE.tile([P, TQ], f32r, tag="exp", name="ex")
                            nc.scalar.activation(
                                ex[:],
                                ps_[:],
                                AF.Exp,
                                bias=kb_sb[:, tkc : tkc + 1],
                                scale=SCALE,
                            )
                            nc.tensor.matmul(
                                pd[:],
                                r(ones_sb[:]),
                                r(ex[:]),
                                start=(tkc == 0),
                                stop=(tkc == nchunks - 1),
                            )
                            nc.tensor.matmul(
                                pav[:],
                                r(
                                    vq[h // 4][
                                        :, tkc, (h % 4) * P : (h % 4 + 1) * P
                                    ]
                                ),
                                r(ex[:]),
                                start=(tkc == 0),
                                stop=(tkc == nchunks - 1),
                            )
                        rec = recp.tile([P, TQ], f32, tag="rec", name="rec")
                        with nc.allow_low_precision("softmax denom"):
                            nc.vector.reciprocal(rec[:], pd[:])
                        nc.vector.tensor_tensor(
                            attnT[:, h, :], pav[:], rec[:], ALU.mult
                        )
                    # output projection for this token tile
                    for nt in range(NHID):
                        wts = []
                        for half in range(2):
                            wt = wop.tile([P, 4, TQ], f32r, tag="wo", name="wo")
                            nc.sync.dma_start(
                                wt[:], wo[nt, half, :, :, :].bitcast(f32r)
                            )
                            wts.append(wt)
                        for tqc in range(TQ // P):
                            po = psum.tile([P, TQ], f32, tag="mm", name="po")
                            for h in range(HG):
                                nc.tensor.matmul(
                                    po[:],
                                    r(attnT[:, h, tqc * P : (tqc + 1) * P]),
                                    r(wts[h // 4][:, h % 4, :]),
                                    start=(h == 0),
                                    stop=(h == HG - 1),
                                )
                            ot = outp.tile([P, TQ], f32, tag="osb", name="ot")
                            nc.vector.tensor_copy(ot[:], po[:])
                            nc.sync.dma_start(out[t, nt, tqc, :, :], ot[:])

    nc.finalize()
    return nc


def _get_program():
    if "nc" not in _CACHED:
        _CACHED["nc"] = _build_program()
    return _CACHED["nc"]


def _host_prep(x, wq_a, q_norm_w, wq_b, wkv_a, kv_norm_w, wkv_b, wo,
               attention_mask, positions):
    """Build the 8 per-core input maps.

    All weight tensors are host-packed partition-major so every device
    DMA is one large contiguous transfer.
    """
    f = np.float32
    x = np.asarray(x, f)
    wq_a = np.asarray(wq_a, f)
    wkv_a = np.asarray(wkv_a, f)
    wq_b3 = np.asarray(wq_b, f).reshape(Q_RANK, H, D_QK)
    wkv_b3 = np.asarray(wkv_b, f).reshape(KV_RANK, H, D_NOPE + D_V)
    wo2 = np.asarray(wo, f)
    q_norm_w = np.asarray(q_norm_w, f)
    kv_norm_w = np.asarray(kv_norm_w, f)
    attention_mask = np.asarray(attention_mask)
    positions = np.asarray(positions)

    qw = np.ascontiguousarray(q_norm_w.reshape(QCH, P).T)
    kvw = np.ascontiguousarray(kv_norm_w.reshape(KCH, P).T)

    # stage-A weights: [block, quart, p, kk, width] (replicated on all cores)
    wqa_blk = np.ascontiguousarray(
        wq_a.reshape(4, KQ, P, 6, 256).transpose(3, 0, 2, 1, 4))
    wkva_blk = np.ascontiguousarray(
        wkv_a[:, :KV_RANK].reshape(4, KQ, P, 2, 256).transpose(3, 0, 2, 1, 4))
    wkvr_blk = np.ascontiguousarray(
        wkv_a[:, KV_RANK:].reshape(4, KQ, P, 64).transpose(0, 2, 1, 3))

    inv_freq = 1.0 / (THETA ** (np.arange(0, D_ROPE, 2, dtype=np.float64) / D_ROPE))

    # sliding causal mask: cm[dk, u] = 0 iff dk <= u - 384
    dk = np.arange(P)[:, None]
    u = np.arange(7 * P)[None, :]
    cmask = np.where(dk <= u - 3 * P, 0.0, NMASK).astype(f)
    onesd = np.ones((P, P), f)

    per_batch = {}
    for b in range(B):
        xTb = np.ascontiguousarray(x[b].T)
        ang = positions[b].astype(np.float64)[:, None] * inv_freq[None, :]
        cosT = np.cos(ang).astype(f).T  # [32, T]
        sinT = np.sin(ang).astype(f).T
        cos4 = np.ascontiguousarray(np.tile(cosT, (4, 1)))
        sin4 = np.ascontiguousarray(
            np.concatenate([-sinT, sinT, -sinT, sinT], axis=0))
        kb = np.where(attention_mask[b] != 0, 0.0, NMASK).astype(f)
        kbias = np.ascontiguousarray(kb.reshape(TC, P).T)
        per_batch[b] = (xTb, cos4, sin4, kbias)

    in_maps = []
    for c in range(8):
        b, g = c // 4, c % 4
        hs = slice(g * HG, (g + 1) * HG)
        xTb, cos4, sin4, kbias = per_batch[b]
        # [rank, head, dim] -> [h, p, k, c] / [pair, p, k, c]
        wqbn_ = np.ascontiguousarray(
            wq_b3[:, hs, :D_NOPE]
            .reshape(QCH, P, HG, P).transpose(2, 1, 0, 3))
        # rope cols packed in head pairs: [h_even 64 | h_odd 64] per 128-col
        wqbr_ = np.ascontiguousarray(
            wq_b3[:, hs, D_NOPE:]
            .reshape(QCH, P, HG // 2, P).transpose(2, 1, 0, 3))
        wkbn_ = np.ascontiguousarray(
            wkv_b3[:, hs, :D_NOPE]
            .reshape(KCH, P, HG, P).transpose(1, 2, 0, 3))
        wkbv_ = np.ascontiguousarray(
            wkv_b3[:, hs, D_NOPE:]
            .reshape(KCH, P, 2, TQ).transpose(2, 1, 0, 3))
        # wo rows h*128+dv, cols nt*512+c -> [nt, half, dv(p), h%4, c]
        wosh = np.ascontiguousarray(
            wo2[g * HG * D_V : (g + 1) * HG * D_V, :]
            .reshape(2, 4, P, NHID, TQ).transpose(3, 0, 2, 1, 4))
        in_maps.append({
            "xT": xTb, "wqa": wqa_blk, "wkva": wkva_blk, "wkvr": wkvr_blk,
            "wqbn": wqbn_, "wqbr": wqbr_, "wkbn": wkbn_, "wkbv": wkbv_,
            "wo": wosh, "qw": qw, "kvw": kvw,
            "cos4": cos4, "sin4": sin4,
            "cmask": cmask, "kbias": kbias, "onesd": onesd,
        })
    return in_maps


def kernel(**inputs):
    from concourse.bass_utils import run_bass_kernel_spmd

    nc = _get_program()
    in_maps = _host_prep(**inputs)
    res = run_bass_kernel_spmd(nc, in_maps, core_ids=list(range(8)))
    _CACHED["last_result"] = res
    out = np.zeros((B, T, HID), np.float32)
    for c in range(8):
        blk = res.results[c]["out"]  # [NT, NHID, 4, P, TQ]
        # row = t*512 + q*128 + p, col = nt*512 + c
        out[c // 4] += blk.transpose(0, 2, 3, 1, 4).reshape(T, HID)
    return out


# revision 27
# speedup vs baseline: 1.3757x; 1.0026x over previous
"""DeepseekV3 MLA forward on 8 TRN2 NeuronCores.

Sharding: data-parallel over batch (B=2 -> 2 groups of 4 cores), tensor-
parallel over heads within each batch group (32 heads -> 4 groups of 8).
Each core computes its batch element's full latent projections (wq_a /
wkv_a replicated), its 8 heads' q/k/v expansions + attention, and a
partial output projection (wo row-shard); the host sums the 4 partial
outputs per batch element.

Dataflow on device keeps activations transposed ([feature, token]) so
every matmul contracts over the partition dim with no on-device
transposes anywhere:
  latentT = wq_a.T @ xT        (lhsT=wq_a cols, rhs=xT)        [rank, T]
  qT_h    = wq_b_h.T @ latentT                                  [d, T]
  scoresT = kT_h-chunks @ qT_h                                  [tk, tq]
  softmax over tk (=partitions): exp on ACT, denominator via a
  ones[128,128] matmul (result replicated across partitions), then
  attn_outT = v_chunks.T @ expT                                 [dv, tq]
  out      = attnT-chunks.T @ wo_h  (natural layout)            [tq, hid]
RoPE in transposed layout: rot(x) = x*cos + swap32(x)*(+-sin), where
swap32 exchanges the two 32-row halves of each 64-row rope block (done
with SBUF->SBUF block DMAs) and the +-sin sign pattern is host-built.
All matmuls run as float32r (FP22-truncated fp32, single pass).

Perf notes: per-DMA issue costs ~1us on the sync sequencer, so weights
are host-packed into large partition-major blocks and DMA'd in few big
transfers (SBUF slices feed the matmuls). SBUF pools are statically
reserved; they are scoped per stage, long-lived activation pools on the
"right" allocator side. PSUM: 8 banks split mm:4 / pd:2 / sc:2.
"""

import os
import sys

import numpy as np

sys.path.insert(0, "/opt/trn_rl_repo")

B, T, HID = 2, 1024, 4096
H, D_NOPE, D_ROPE, D_V = 32, 128, 64, 128
D_QK = D_NOPE + D_ROPE
Q_RANK, KV_RANK = 1536, 512
THETA, EPS = 10000.0, 1e-6
SCALE = float(D_QK) ** -0.5
NMASK = -30000.0

HG = H // 4          # heads per core = 8
P = 128
QCH = Q_RANK // P    # 12 latent chunks (q)
KCH = KV_RANK // P   # 4 latent chunks (kv)
HIDK = HID // P      # 32 contraction tiles for stage A
KQ = HIDK // 4       # 8 k-tiles per stage-A quarter
TQ = 512             # token tile (free dim) for most matmuls
NT = T // TQ         # 2 token tiles
TC = T // P          # 8 token chunks of 128
NHID = HID // TQ     # 8 output column tiles

_CACHED = {}
STAGE_MARKS = []


def _build_program():
    import contextlib

    import concourse.bacc as bacc
    import concourse.mybir as mybir
    import concourse.tile as tile

    f32 = mybir.dt.float32
    f32r = mybir.dt.float32r
    AF = mybir.ActivationFunctionType
    ALU = mybir.AluOpType

    nc = bacc.Bacc()

    # ---- DRAM I/O (per-core shapes; SPMD across the 8 cores) ----
    # weights are host-packed partition-major so each DMA is one big
    # contiguous transfer whose SBUF image is sliced per matmul
    xT = nc.dram_tensor("xT", (HID, T), f32, kind="ExternalInput")
    wqa = nc.dram_tensor("wqa", (6, 4, P, KQ, 256), f32, kind="ExternalInput")
    wkva = nc.dram_tensor("wkva", (2, 4, P, KQ, 256), f32, kind="ExternalInput")
    wkvr = nc.dram_tensor("wkvr", (4, P, KQ, 64), f32, kind="ExternalInput")
    wqbn = nc.dram_tensor("wqbn", (HG, P, QCH, P), f32, kind="ExternalInput")
    wqbr = nc.dram_tensor("wqbr", (HG // 2, P, QCH, P), f32, kind="ExternalInput")
    wkbn = nc.dram_tensor("wkbn", (P, HG, KCH, P), f32, kind="ExternalInput")
    wkbv = nc.dram_tensor("wkbv", (2, P, KCH, TQ), f32, kind="ExternalInput")
    wo = nc.dram_tensor("wo", (NHID, 2, P, 4, TQ), f32, kind="ExternalInput")
    qw = nc.dram_tensor("qw", (P, QCH), f32, kind="ExternalInput")
    kvw = nc.dram_tensor("kvw", (P, KCH), f32, kind="ExternalInput")
    cos4 = nc.dram_tensor("cos4", (P, T), f32, kind="ExternalInput")
    sin4 = nc.dram_tensor("sin4", (P, T), f32, kind="ExternalInput")  # +-sin
    cmask = nc.dram_tensor("cmask", (P, 7 * P), f32, kind="ExternalInput")
    kbias = nc.dram_tensor("kbias", (P, TC), f32, kind="ExternalInput")
    onesd = nc.dram_tensor("onesd", (P, P), f32, kind="ExternalInput")
    out = nc.dram_tensor("out", (NT, NHID, 4, P, TQ), f32, kind="ExternalOutput")

    def r(ap):
        return ap.bitcast(f32r)

    with tile.TileContext(nc) as tc, contextlib.ExitStack() as rstack:
        with (
            tc.tile_pool(name="const", bufs=1) as const,
            tc.tile_pool(name="psmm", bufs=3, space="PSUM") as psum,
            tc.tile_pool(name="pspd", bufs=2, space="PSUM") as pspd,
            tc.tile_pool(name="pssc", bufs=3, space="PSUM") as pssc,
        ):
            # ---- constants (persistent; DMAs deferred past the first
            # stage-A tiles so they don't delay the first matmuls) ----
            ones_sb = const.tile([P, P], f32r, tag="ones")
            qw_sb = const.tile([P, QCH], f32, tag="qw")
            kvw_sb = const.tile([P, KCH], f32, tag="kvw")
            cos_sb = const.tile([P, T], f32, tag="cos")
            sin_sb = const.tile([P, T], f32, tag="sin")
            kb_sb = const.tile([P, TC], f32, tag="kb")
            zero_b = const.tile([P, 1], f32, tag="zb")
            nc.vector.memset(zero_b[:], 0.0)
            eps_b = const.tile([P, 1], f32, tag="eb")
            nc.vector.memset(eps_b[:], EPS)

            def emit_const_dmas():
                nc.sync.dma_start(ones_sb[:], onesd[:, :].bitcast(f32r))
                nc.sync.dma_start(qw_sb[:], qw[:, :])
                nc.sync.dma_start(kvw_sb[:], kvw[:, :])
                nc.sync.dma_start(cos_sb[:], cos4[:, :])
                nc.sync.dma_start(sin_sb[:], sin4[:, :])
                nc.sync.dma_start(kb_sb[:], kbias[:, :])

            def rmsnorm(lat, nch, w_sb, fan, sspool, sstag, wrk):
                for t in range(NT):
                    ssp = sspool.tile([P, TQ], f32, tag=sstag, name="ssp")
                    for m in range(nch):
                        sq = wrk.tile([P, TQ], f32r, tag="sq", name="sq")
                        nc.scalar.activation(
                            sq[:],
                            lat[m][:, t * TQ : (t + 1) * TQ],
                            AF.Square,
                            bias=zero_b[:],
                        )
                        nc.tensor.matmul(
                            ssp[:],
                            r(ones_sb[:]),
                            r(sq[:]),
                            start=(m == 0),
                            stop=(m == nch - 1),
                        )
                    std = wrk.tile([P, TQ], f32, tag="std", name="std")
                    nc.scalar.activation(
                        std[:], ssp[:], AF.Sqrt, bias=eps_b[:], scale=1.0 / fan
                    )
                    rstd = wrk.tile([P, TQ], f32, tag="rstd", name="rstd")
                    with nc.allow_low_precision("rmsnorm rstd"):
                        nc.vector.reciprocal(rstd[:], std[:])
                    for m in range(nch):
                        sl = lat[m][:, t * TQ : (t + 1) * TQ]
                        nc.vector.tensor_tensor(sl, sl, rstd[:], ALU.mult)
                        nc.vector.tensor_scalar_mul(sl, sl, w_sb[:, m : m + 1])

            with tc.tile_pool(name="wqb0p", bufs=1) as wqb0p, \
                    tc.tile_pool(name="qlatp", bufs=1) as qlatp:
                wqb0 = wqb0p.tile([P, QCH, P], f32r, tag="wqb0")
                qlat = [
                    qlatp.tile([P, T], f32r, tag=f"qlat{i}", name=f"qlat{i}")
                    for i in range(QCH)
                ]
                with tc.tile_pool(name="kvlatp", bufs=1) as kvlatp:
                    kvlat = [
                        kvlatp.tile([P, T], f32r, tag=f"kvlat{i}", name=f"kvlat{i}")
                        for i in range(KCH + 1)
                    ]
                    with (
                        tc.tile_pool(name="wkbnp", bufs=1) as wkbnp,
                        tc.tile_pool(name="wkbvp", bufs=1) as wkbvp,
                    ):
                        # stage-D weights: pool reserved up front (no
                        # overlap deps); DMAs emitted mid-stage-A so they
                        # neither delay the first x tiles nor stall D
                        wkn = wkbnp.tile(
                            [P, HG, KCH, P], f32r, tag="wkbn", name="wkbn"
                        )
                        wkvts = [
                            wkbvp.tile(
                                [P, KCH, TQ], f32r, tag=f"wkbv{quad}", name="wkbv"
                            )
                            for quad in range(2)
                        ]

                        def emit_dweight_dmas():
                            nc.sync.dma_start(
                                wkn[:], wkbn[:, :, :, :].bitcast(f32r))
                            for quad in range(2):
                                nc.sync.dma_start(
                                    wkvts[quad][:],
                                    wkbv[quad, :, :, :].bitcast(f32r),
                                )
                            nc.sync.dma_start(
                                wqb0[:], wqbn[0, :, :, :].bitcast(f32r))

                        # ---- stage A: latent projections ----
                        # kv blocks first so the kv path unblocks early
                        # kpe2 lives to the end (right side)
                        kpep = rstack.enter_context(
                            tc.tile_pool(name="kpep", bufs=1, side="right"))
                        kpe2 = kpep.tile([P, T], f32r, tag="kpe2")

                        kvblocks = [
                            (wkva, 0, 256, kvlat, 0),
                            (wkva, 1, 256, kvlat, 2),
                            (wkvr, None, 64, kvlat, 4),
                        ]
                        qblocks = [
                            (wqa, mb, 256, qlat, 2 * mb)
                            for mb in range(QCH // 2)
                        ]

                        wrkA_cm = tc.tile_pool(name="wrkA", bufs=3)
                        wrkA = wrkA_cm.__enter__()
                        with (
                            tc.tile_pool(name="xk", bufs=10) as xkp,
                            tc.tile_pool(name="wA", bufs=3) as wAp,
                        ):
                            for quart in range(4):
                                xk = [None] * KQ
                                mblocks = kvblocks + qblocks
                                for mbi, (wdram, blki, width, dest, dch) in \
                                        enumerate(mblocks):
                                    nm = (width + P - 1) // P
                                    wt = wAp.tile(
                                        [P, KQ, 256], f32r, tag="wA", name="wA"
                                    )
                                    wsrc = (
                                        wdram[quart, :, :, :]
                                        if blki is None
                                        else wdram[blki, quart, :, :, :]
                                    )
                                    first = quart == 0 and mbi <= 1
                                    if mbi == 0:
                                        # interleave x and weight loads
                                        for kk in range(KQ):
                                            k = quart * KQ + kk
                                            xt_ = xkp.tile(
                                                [P, T], f32r, tag="xk", name="xk"
                                            )
                                            nc.sync.dma_start(
                                                xt_[:],
                                                xT[k * P : (k + 1) * P, :].bitcast(
                                                    f32r
                                                ),
                                            )
                                            xk[kk] = xt_
                                            if first:
                                                # fine-grained first weight
                                                # loads so the kernel starts
                                                # computing immediately
                                                nc.sync.dma_start(
                                                    wt[:, kk, :width],
                                                    wsrc[:, kk, :].bitcast(f32r),
                                                )
                                    if first and mbi == 1:
                                        for kk in range(KQ):
                                            nc.sync.dma_start(
                                                wt[:, kk, :width],
                                                wsrc[:, kk, :].bitcast(f32r),
                                            )
                                    if not first:
                                        nc.sync.dma_start(
                                            wt[:, :, :width], wsrc.bitcast(f32r)
                                        )
                                    pst = [
                                        [
                                            (psum if (mbi + m_) % 2 == 0 else pssc)
                                            .tile(
                                                [P, TQ],
                                                f32,
                                                tag=(
                                                    "mm"
                                                    if (mbi + m_) % 2 == 0
                                                    else "psc"
                                                ),
                                                name="psA",
                                            )
                                            for _ in range(NT)
                                        ]
                                        for m_ in range(nm)
                                    ]
                                    for m in range(nm):
                                        w = min(width - m * P, P)
                                        for kk in range(KQ):
                                            for t in range(NT):
                                                nc.tensor.matmul(
                                                    pst[m][t][:w, :],
                                                    r(wt[:, kk, m * P : m * P + w]),
                                                    r(
                                                        xk[kk][
                                                            :,
                                                            t * TQ : (t + 1) * TQ,
                                                        ]
                                                    ),
                                                    start=(kk == 0),
                                                    stop=(kk == KQ - 1),
                                                )
                                        for t in range(NT):
                                            dst = dest[dch + m][
                                                :w, t * TQ : (t + 1) * TQ
                                            ]
                                            if quart == 0:
                                                nc.scalar.copy(dst, pst[m][t][:w, :])
                                            else:
                                                nc.vector.tensor_tensor(
                                                    dst,
                                                    pst[m][t][:w, :],
                                                    dst,
                                                    ALU.add,
                                                )
                                    if quart == 0 and mbi == 0:
                                        emit_const_dmas()
                                    if quart == 1 and mbi == 0:
                                        emit_dweight_dmas()
                                    if quart == 3 and mbi == len(kvblocks) - 1:
                                        # kv latents final: norm + k rope now
                                        # so the kv path completes during the
                                        # remaining q blocks
                                        STAGE_MARKS.append(("A2kv", nc.next_id()))
                                        rmsnorm(kvlat, KCH, kvw_sb, KV_RANK,
                                                pspd, "pd", wrkA)
                                        ksw = xkp.tile(
                                            [P, T], f32r, tag="xk", name="ksw"
                                        )
                                        kswf = ksw[:].bitcast(f32)
                                        nc.sync.dma_start(
                                            kpe2[0:64, :], kvlat[KCH][0:64, :])
                                        nc.sync.dma_start(
                                            kpe2[64:128, :], kvlat[KCH][0:64, :])
                                        nc.sync.dma_start(
                                            kswf[0:32, :],
                                            kvlat[KCH][32:64, :].bitcast(f32))
                                        nc.sync.dma_start(
                                            kswf[32:64, :],
                                            kvlat[KCH][0:32, :].bitcast(f32))
                                        nc.sync.dma_start(
                                            kswf[64:96, :],
                                            kvlat[KCH][32:64, :].bitcast(f32))
                                        nc.sync.dma_start(
                                            kswf[96:128, :],
                                            kvlat[KCH][0:32, :].bitcast(f32))
                                        nc.vector.tensor_tensor(
                                            kswf, kswf, sin_sb[:], ALU.mult)
                                        nc.vector.tensor_tensor(
                                            kpe2[:], kpe2[:], cos_sb[:], ALU.mult)
                                        nc.vector.tensor_tensor(
                                            kpe2[:], kpe2[:], kswf, ALU.add)


                        kTnp = rstack.enter_context(
                            tc.tile_pool(name="kTnp", bufs=1, side="right"))
                        kTn = kTnp.tile([P, HG, T], f32r, tag="kTn")
                        vqp = rstack.enter_context(
                            tc.tile_pool(name="vqp", bufs=2, side="right"))
                        vq = [
                            vqp.tile([P, TC, 4 * D_V], f32r, tag="vq", name="vq")
                            for _ in range(2)
                        ]

                        STAGE_MARKS.append(("D", nc.next_id()))
                        # ---- stage D: kT_nope per head, v per quad ----
                        for h in range(HG):
                            pp, ptag = (psum, "mm") if h % 2 == 0 else (pssc, "psc")
                            pst = [
                                pp.tile([P, TQ], f32, tag=ptag, name="psD")
                                for _ in range(NT)
                            ]
                            for k in range(KCH):
                                for t in range(NT):
                                    nc.tensor.matmul(
                                        pst[t][:],
                                        r(wkn[:, h, k, :]),
                                        r(kvlat[k][:, t * TQ : (t + 1) * TQ]),
                                        start=(k == 0),
                                        stop=(k == KCH - 1),
                                    )
                            for t in range(NT):
                                nc.vector.tensor_copy(
                                    kTn[:, h, t * TQ : (t + 1) * TQ], pst[t][:]
                                )
                        for quad in range(2):
                            for tkc in range(TC):
                                pp, ptag = (
                                    (psum, "mm") if tkc % 2 == 0 else (pssc, "psc")
                                )
                                ps_ = pp.tile([P, TQ], f32, tag=ptag, name="psV")
                                for k in range(KCH):
                                    nc.tensor.matmul(
                                        ps_[:],
                                        r(kvlat[k][:, tkc * P : (tkc + 1) * P]),
                                        r(wkvts[quad][:, k, :]),
                                        start=(k == 0),
                                        stop=(k == KCH - 1),
                                    )
                                nc.vector.tensor_copy(vq[quad][:, tkc, :], ps_[:])

                        STAGE_MARKS.append(("A2q", nc.next_id()))
                        rmsnorm(qlat, QCH, qw_sb, Q_RANK, pssc, "psc", wrkA)
                        wrkA_cm.__exit__(None, None, None)

                # kvlat + stage-D weight pools closed here
                actq = rstack.enter_context(
                    tc.tile_pool(name="actq", bufs=1, side="right"))
                qTn = actq.tile([P, HG, T], f32r, tag="qTn")
                qTr = actq.tile([P, HG // 2, T], f32r, tag="qTr")

                STAGE_MARKS.append(("B", nc.next_id()))
                # ---- stage B: qT per head (one DMA per head/pair) ----
                with (
                    tc.tile_pool(name="wqbp", bufs=3) as wqbp,
                    tc.tile_pool(name="qswp", bufs=1) as qswp,
                ):
                    for h in range(HG):
                        if h == 0:
                            wt = wqb0
                        else:
                            wt = wqbp.tile(
                                [P, QCH, P], f32r, tag="wqb", name="wqbn")
                            nc.sync.dma_start(
                                wt[:], wqbn[h, :, :, :].bitcast(f32r))
                        pp, ptag = (psum, "mm") if h % 2 == 0 else (pssc, "psc")
                        pst = [
                            pp.tile([P, TQ], f32, tag=ptag, name="psB")
                            for _ in range(NT)
                        ]
                        for k in range(QCH):
                            for t in range(NT):
                                nc.tensor.matmul(
                                    pst[t][:],
                                    r(wt[:, k, :]),
                                    r(qlat[k][:, t * TQ : (t + 1) * TQ]),
                                    start=(k == 0),
                                    stop=(k == QCH - 1),
                                )
                        for t in range(NT):
                            nc.vector.tensor_copy(
                                qTn[:, h, t * TQ : (t + 1) * TQ], pst[t][:]
                            )
                    for p_ in range(HG // 2):
                        wt = wqbp.tile([P, QCH, P], f32r, tag="wqb", name="wqbr")
                        nc.sync.dma_start(wt[:], wqbr[p_, :, :, :].bitcast(f32r))
                        pp, ptag = (psum, "mm") if p_ % 2 == 0 else (pssc, "psc")
                        pst = [
                            pp.tile([P, TQ], f32, tag=ptag, name="psB2")
                            for _ in range(NT)
                        ]
                        for k in range(QCH):
                            for t in range(NT):
                                nc.tensor.matmul(
                                    pst[t][:],
                                    r(wt[:, k, :]),
                                    r(qlat[k][:, t * TQ : (t + 1) * TQ]),
                                    start=(k == 0),
                                    stop=(k == QCH - 1),
                                )
                        for t in range(NT):
                            nc.vector.tensor_copy(
                                qTr[:, p_, t * TQ : (t + 1) * TQ], pst[t][:]
                            )
                        # rope this pair immediately (overlaps next pair)
                        qsw = qswp.tile([P, T], f32, tag="qsw", name="qsw")
                        qp = qTr[:, p_, :]
                        nc.sync.dma_start(qsw[0:32, :], qp[32:64, :].bitcast(f32))
                        nc.sync.dma_start(qsw[32:64, :], qp[0:32, :].bitcast(f32))
                        nc.sync.dma_start(qsw[64:96, :], qp[96:128, :].bitcast(f32))
                        nc.sync.dma_start(qsw[96:128, :], qp[64:96, :].bitcast(f32))
                        nc.vector.tensor_tensor(qsw[:], qsw[:], sin_sb[:], ALU.mult)
                        nc.vector.tensor_tensor(qp, qp, cos_sb[:], ALU.mult)
                        nc.vector.tensor_tensor(qp, qp, qsw[:], ALU.add)

            STAGE_MARKS.append(("EF", nc.next_id()))
            # kvlat + qlat pools closed here
            # ---- stages E+F per token tile (t=1 first: its leading tk
            # chunks need no causal mask, hiding the mask DMA) ----
            with (
                tc.tile_pool(name="cmp", bufs=1) as cmp_,
                tc.tile_pool(name="attp", bufs=1) as attp,
                tc.tile_pool(name="wrkE", bufs=3) as wrkE,
                tc.tile_pool(name="recp", bufs=2) as recp,
                tc.tile_pool(name="wop", bufs=3) as wop,
                tc.tile_pool(name="outs", bufs=2) as outp,
            ):
                # sliding causal mask: cm[dk, u] = 0 iff dk <= u - 384;
                # chunk variant rv uses columns [(3-rv)*128, (3-rv)*128+512)
                cm_sb = cmp_.tile([P, 7 * P], f32, tag="cm")
                nc.sync.dma_start(cm_sb[:], cmask[:, :])
                for t in (1, 0):
                    attnT = attp.tile([P, HG, TQ], f32r, tag="attnT", name="attnT")
                    nchunks = 4 * (t + 1)
                    for h in range(HG):
                        hb = 64 * (h % 2)
                        pd = pspd.tile([P, TQ], f32, tag="pd", name="pd")
                        pav = psum.tile([P, TQ], f32, tag="mm", name="pav")
                        for tkc in range(nchunks):
                            ps_ = pssc.tile([P, TQ], f32, tag="psc", name="psc")
                            nc.tensor.matmul(
                                ps_[:],
                                r(kTn[:, h, tkc * P : (tkc + 1) * P]),
                                r(qTn[:, h, t * TQ : (t + 1) * TQ]),
                                start=True,
                                stop=False,
                            )
                            nc.tensor.matmul(
                                ps_[:],
                                r(kpe2[hb : hb + 64, tkc * P : (tkc + 1) * P]),
                                r(
                                    qTr[
                                        hb : hb + 64,
                                        h // 2,
                                        t * TQ : (t + 1) * TQ,
                                    ]
                                ),
                                start=False,
                                stop=True,
                            )
                            if tkc >= 4 * t:
                                rv = tkc - 4 * t
                                nc.vector.tensor_tensor(
                                    ps_[:],
                                    ps_[:],
                                    cm_sb[:, (3 - rv) * P : (3 - rv) * P + TQ],
                                    ALU.add,
                                )
                            ex = wrkE.tile([P, TQ], f32r, tag="exp", name="ex")
                            nc.scalar.activation(
                                ex[:],
                                ps_[:],
                                AF.Exp,
                                bias=kb_sb[:, tkc : tkc + 1],
                                scale=SCALE,
                            )
                            nc.tensor.matmul(
                                pd[:],
                                r(ones_sb[:]),
                                r(ex[:]),
                                start=(tkc == 0),
                                stop=(tkc == nchunks - 1),
                            )
                            nc.tensor.matmul(
                                pav[:],
                                r(
                                    vq[h // 4][
                                        :, tkc, (h % 4) * P : (h % 4 + 1) * P
                                    ]
                                ),
                                r(ex[:]),
                                start=(tkc == 0),
                                stop=(tkc == nchunks - 1),
                            )
                        rec = recp.tile([P, TQ], f32, tag="rec", name="rec")
                        with nc.allow_low_precision("softmax denom"):
                            nc.vector.reciprocal(rec[:], pd[:])
                        nc.vector.tensor_tensor(
                            attnT[:, h, :], pav[:], rec[:], ALU.mult
                        )
                    # output projection for this token tile
                    for nt in range(NHID):
                        wts = []
                        for half in range(2):
                            wt = wop.tile([P, 4, TQ], f32r, tag="wo", name="wo")
                            nc.sync.dma_start(
                                wt[:], wo[nt, half, :, :, :].bitcast(f32r)
                            )
                            wts.append(wt)
                        for tqc in range(TQ // P):
                            po = psum.tile([P, TQ], f32, tag="mm", name="po")
                            for h in range(HG):
                                nc.tensor.matmul(
                                    po[:],
                                    r(attnT[:, h, tqc * P : (tqc + 1) * P]),
                                    r(wts[h // 4][:, h % 4, :]),
                                    start=(h == 0),
                                    stop=(h == HG - 1),
                                )
                            ot = outp.tile([P, TQ], f32, tag="osb", name="ot")
                            nc.vector.tensor_copy(ot[:], po[:])
                            nc.sync.dma_start(out[t, nt, tqc, :, :], ot[:])

    nc.finalize()
    return nc


def _get_program():
    if "nc" not in _CACHED:
        _CACHED["nc"] = _build_program()
    return _CACHED["nc"]


def _host_prep(x, wq_a, q_norm_w, wq_b, wkv_a, kv_norm_w, wkv_b, wo,
               attention_mask, positions):
    """Build the 8 per-core input maps.

    All weight tensors are host-packed partition-major so every device
    DMA is one large contiguous transfer.
    """
    f = np.float32
    x = np.asarray(x, f)
    wq_a = np.asarray(wq_a, f)
    wkv_a = np.asarray(wkv_a, f)
    wq_b3 = np.asarray(wq_b, f).reshape(Q_RANK, H, D_QK)
    wkv_b3 = np.asarray(wkv_b, f).reshape(KV_RANK, H, D_NOPE + D_V)
    wo2 = np.asarray(wo, f)
    q_norm_w = np.asarray(q_norm_w, f)
    kv_norm_w = np.asarray(kv_norm_w, f)
    attention_mask = np.asarray(attention_mask)
    positions = np.asarray(positions)

    qw = np.ascontiguousarray(q_norm_w.reshape(QCH, P).T)
    kvw = np.ascontiguousarray(kv_norm_w.reshape(KCH, P).T)

    # stage-A weights: [block, quart, p, kk, width] (replicated on all cores)
    wqa_blk = np.ascontiguousarray(
        wq_a.reshape(4, KQ, P, 6, 256).transpose(3, 0, 2, 1, 4))
    wkva_blk = np.ascontiguousarray(
        wkv_a[:, :KV_RANK].reshape(4, KQ, P, 2, 256).transpose(3, 0, 2, 1, 4))
    wkvr_blk = np.ascontiguousarray(
        wkv_a[:, KV_RANK:].reshape(4, KQ, P, 64).transpose(0, 2, 1, 3))

    inv_freq = 1.0 / (THETA ** (np.arange(0, D_ROPE, 2, dtype=np.float64) / D_ROPE))

    # sliding causal mask: cm[dk, u] = 0 iff dk <= u - 384
    dk = np.arange(P)[:, None]
    u = np.arange(7 * P)[None, :]
    cmask = np.where(dk <= u - 3 * P, 0.0, NMASK).astype(f)
    onesd = np.ones((P, P), f)

    per_batch = {}
    for b in range(B):
        xTb = np.ascontiguousarray(x[b].T)
        ang = positions[b].astype(np.float64)[:, None] * inv_freq[None, :]
        cosT = np.cos(ang).astype(f).T  # [32, T]
        sinT = np.sin(ang).astype(f).T
        cos4 = np.ascontiguousarray(np.tile(cosT, (4, 1)))
        sin4 = np.ascontiguousarray(
            np.concatenate([-sinT, sinT, -sinT, sinT], axis=0))
        kb = np.where(attention_mask[b] != 0, 0.0, NMASK).astype(f)
        kbias = np.ascontiguousarray(kb.reshape(TC, P).T)
        per_batch[b] = (xTb, cos4, sin4, kbias)

    in_maps = []
    for c in range(8):
        b, g = c // 4, c % 4
        hs = slice(g * HG, (g + 1) * HG)
        xTb, cos4, sin4, kbias = per_batch[b]
        # [rank, head, dim] -> [h, p, k, c] / [pair, p, k, c]
        wqbn_ = np.ascontiguousarray(
            wq_b3[:, hs, :D_NOPE]
            .reshape(QCH, P, HG, P).transpose(2, 1, 0, 3))
        # rope cols packed in head pairs: [h_even 64 | h_odd 64] per 128-col
        wqbr_ = np.ascontiguousarray(
            wq_b3[:, hs, D_NOPE:]
            .reshape(QCH, P, HG // 2, P).transpose(2, 1, 0, 3))
        wkbn_ = np.ascontiguousarray(
            wkv_b3[:, hs, :D_NOPE]
            .reshape(KCH, P, HG, P).transpose(1, 2, 0, 3))
        wkbv_ = np.ascontiguousarray(
            wkv_b3[:, hs, D_NOPE:]
            .reshape(KCH, P, 2, TQ).transpose(2, 1, 0, 3))
        # wo rows h*128+dv, cols nt*512+c -> [nt, half, dv(p), h%4, c]
        wosh = np.ascontiguousarray(
            wo2[g * HG * D_V : (g + 1) * HG * D_V, :]
            .reshape(2, 4, P, NHID, TQ).transpose(3, 0, 2, 1, 4))
        in_maps.append({
            "xT": xTb, "wqa": wqa_blk, "wkva": wkva_blk, "wkvr": wkvr_blk,
            "wqbn": wqbn_, "wqbr": wqbr_, "wkbn": wkbn_, "wkbv": wkbv_,
            "wo": wosh, "qw": qw, "kvw": kvw,
            "cos4": cos4, "sin4": sin4,
            "cmask": cmask, "kbias": kbias, "onesd": onesd,
        })
    return in_maps


def kernel(**inputs):
    from concourse.bass_utils import run_bass_kernel_spmd

    nc = _get_program()
    in_maps = _host_prep(**inputs)
    res = run_bass_kernel_spmd(nc, in_maps, core_ids=list(range(8)))
    _CACHED["last_result"] = res
    out = np.zeros((B, T, HID), np.float32)
    for c in range(8):
        blk = res.results[c]["out"]  # [NT, NHID, 4, P, TQ]
        # row = t*512 + q*128 + p, col = nt*512 + c
        out[c // 4] += blk.transpose(0, 2, 3, 1, 4).reshape(T, HID)
    return out
